# revision 1
# baseline (speedup 1.0000x reference)
"""ABC attention (gated slot attention) on 8 TRN2 NeuronCores.

Sharding: 2 heads per core (16 heads / 8 cores). Per core:
  - projections q,k (RoPE, q pre-scaled), v, silu(gate), slot logits
    (hs.T resident in SBUF f32r; each output accumulates over all 16
    d-chunks in PSUM, drained through RoPE/silu directly from PSUM)
  - quadratic chunked ABC attention; all big matmuls f32r with moving
    dim 512 (full PE rate); causal masking via constant mask tiles
  - fused RMS-norm x gate epilogue in [dv, t] layout
  - AllToAll reshards o_g head-split -> T-split; per-core o_proj over
    its 256-row T slice; host concatenates core outputs.
"""
import sys
if '/opt/trn_rl_repo' not in sys.path:
    sys.path.insert(0, '/opt/trn_rl_repo')
import numpy as np

import concourse.bacc as bacc
import concourse.mybir as mybir
import concourse.tile as tile
from concourse import bass_utils

F32 = mybir.dt.float32
F32R = mybir.dt.float32r
BF16 = mybir.dt.bfloat16
AF = mybir.ActivationFunctionType

H, DK, DV, M, T, D = 16, 128, 128, 64, 2048, 2048
EPS, CLAMP, ROPE_BASE = 1e-5, 32.0, 10000.0
N_CORES = 8
NT = T // 128        # 16
NB = T // 512        # 4 big row-chunks
ND = D // 128        # 16
SCALE = DK ** -0.5

REPEAT = 1
DEBUG = False
NOCC = False  # replace AllGather with local copy (timing/timeline diagnostics)


def build(repeat=1, debug=False, nocc=False):
    nc = bacc.Bacc(None, target_bir_lowering=False, debug=False, num_devices=N_CORES)

    din = {}
    for nm, shp, dt in [
        ("hsT", [D, T], F32R),
        ("wq0", [D, 128], F32R), ("wq1", [D, 128], F32R),
        ("wk0", [D, 128], F32R), ("wk1", [D, 128], F32R),
        ("wg0", [D, 128], F32R), ("wg1", [D, 128], F32R),
        ("wvT", [D, 256], F32R), ("ws1T", [D, 16], F32R),
        ("ws2e", [17, 128], F32R),
        ("ones2k", [1, T], F32R),
        ("onesrow", [1, 128], F32R), ("onescol", [128, 1], F32R),
        ("cossin", [128, T], F32),
        ("triu", [128, 128], F32R),
        ("masks", [128, 4, 512], F32), ("ident", [128, 128], F32),
        ("woT", [D, D], BF16),
    ]:
        din[nm] = nc.dram_tensor(nm, shp, dt, kind="ExternalInput")
    out_d = nc.dram_tensor("out", [256, D], F32, kind="ExternalOutput")

    dbg = {}
    if debug:
        for nm, shp in [("qT", [256, T]), ("kT", [256, T]), ("v", [128, NT * 256]),
                        ("sg", [256, T]), ("es", [128, NT * 128]),
                        ("enz", [128, NT * 128]), ("esT", [128, T]),
                        ("ok", [128, T]), ("qveT", [128, T]), ("ogT", [256, T])]:
            dbg[nm] = nc.dram_tensor("dbg_" + nm, shp, F32, kind="ExternalOutput")

    with tile.TileContext(nc) as tc:
        with tc.tile_pool(name="const", bufs=1) as cpool, \
             tc.tile_pool(name="big", bufs=1) as big:
            c = {}
            for nm, dt in [("ws2e", F32R), ("onesrow", F32R), ("onescol", F32R),
                           ("triu", F32R)]:
                tl = cpool.tile(list(din[nm].shape), dt, tag=nm, name=nm)
                nc.sync.dma_start(tl[:], din[nm].ap())
                c[nm] = tl

            st = {
                "qT": [big.tile([128, T], F32R, tag=f"qT{h}", name=f"qT{h}") for h in range(2)],
                "kT": [big.tile([128, T], F32R, tag=f"kT{h}", name=f"kT{h}") for h in range(2)],
                "sg": [big.tile([128, T], F32, tag=f"sg{h}", name=f"sg{h}") for h in range(2)],
                "v": big.tile([128, NT, 256], F32R, tag="v", name="v"),
                "u17": big.tile([17, T], F32R, tag="u17", name="u17"),
                "es_t": big.tile([128, NT, 128], F32R, tag="es_t", name="es_t"),
                "enz": big.tile([128, NT, 128], F32, tag="enz", name="enz"),
                "esT": big.tile([128, T], F32R, tag="esT", name="esT"),
            }
            for _ in range(repeat):
                _pass(nc, tc, din, c, st, out_d, dbg, debug, nocc)

    nc.compile()
    return nc, dbg


def _pass(nc, tc, din, c, st, out_d, dbg, debug, nocc=False):
    qT, kT, sg = st["qT"], st["kT"], st["sg"]
    v, u17, es_t, enz, esT = st["v"], st["u17"], st["es_t"], st["enz"], st["esT"]

    # ================= PHASE 1: projections (T-eighth sweeps) =================
    with tc.tile_pool(name="p1w", bufs=1) as p1w, \
         tc.tile_pool(name="p1sb", bufs=2) as p1sb, \
         tc.tile_pool(name="p1hs", bufs=3) as p1hs:
        p1ps_cm = tc.tile_pool(name="p1ps", bufs=1, space="PSUM")
        p1ps = p1ps_cm.__enter__()
        cossin = p1w.tile([128, T], F32, tag="cossin", name="cossin")
        nc.sync.dma_start(cossin[:], din["cossin"].ap())
        wts = {}
        for wn in ("wq0", "wq1", "wk0", "wk1", "wg0", "wg1", "wvT", "ws1T"):
            cw = din[wn].shape[1]
            wt = p1w.tile([128, ND, cw], F32R, tag=wn, name=wn)
            nc.sync.dma_start(wt[:], din[wn].ap().rearrange("(k p) c -> p k c", p=128))
            wts[wn] = wt
        nc.sync.dma_start(u17[16:17, :], din["ones2k"].ap())

        carries = []
        hsd = din["hsT"].ap().rearrange("(k p) t -> p k t", p=128)
        QKG = (("wq0", qT[0], "q"), ("wq1", qT[1], "q"),
               ("wk0", kT[0], "k"), ("wk1", kT[1], "k"),
               ("wg0", sg[0], "g"), ("wg1", sg[1], "g"))
        for tb in range(8):
            tsl = slice(tb * 256, (tb + 1) * 256)
            hsq = [p1hs.tile([128, 8, 256], F32R, tag="hsq", name=f"hsq{tb}_{qq}")
                   for qq in range(2)]
            for qq in range(2):
                nc.sync.dma_start(hsq[qq][:], hsd[:, qq * 8:(qq + 1) * 8, tsl])
            acc = {wn: p1ps.tile([128, 256], F32, tag=f"acc_{wn}",
                                 name=f"acc_{wn}_{tb}")
                   for wn, *_ in QKG}
            accv = [p1ps.tile([128, 256], F32, tag=f"acc_v{i}", name=f"acc_v{i}_{tb}")
                    for i in range(2)]
            for d in range(ND):
                hs_d = hsq[d // 8][:, d % 8, :]
                for wn, dst, kind in QKG:
                    nc.tensor.matmul(acc[wn][:], wts[wn][:, d, :], hs_d,
                                     start=(d == 0), stop=(d == ND - 1))
                for i in range(2):
                    nc.tensor.matmul(accv[i][:],
                                     hs_d[:, i * 128:(i + 1) * 128],
                                     wts["wvT"][:, d, :],
                                     start=(d == 0), stop=(d == ND - 1))
            # drains
            for wn, dst, kind in QKG:
                ps = acc[wn]
                if kind == "g":
                    sgm = p1sb.tile([128, 256], F32, tag="sgm")
                    nc.scalar.activation(sgm[:], ps[:], AF.Sigmoid)
                    nc.vector.tensor_mul(dst[:, tsl], ps[:], sgm[:])
                else:
                    t1 = p1sb.tile([64, 256], F32, tag="ropet1")
                    t2 = p1sb.tile([64, 256], F32, tag="ropet2")
                    nc.vector.tensor_mul(t1[:], ps[0:64, :], cossin[0:64, tsl])
                    nc.vector.tensor_mul(t2[:], ps[64:128, :], cossin[64:128, tsl])
                    nc.vector.tensor_sub(dst[0:64, tsl], t1[:], t2[:])
                    nc.vector.tensor_mul(t1[:], ps[64:128, :], cossin[0:64, tsl])
                    nc.vector.tensor_mul(t2[:], ps[0:64, :], cossin[64:128, tsl])
                    nc.vector.tensor_add(dst[64:128, tsl], t1[:], t2[:])
            for i in range(2):
                nc.scalar.copy(v[:, 2 * tb + i, :], accv[i][:])
            # u pass (reuses a freed accumulator bank via tag rotation)
            accu = p1ps.tile([16, 256], F32, tag="acc_wq0", name=f"acc_u_{tb}")
            for d in range(ND):
                hs_d = hsq[d // 8][:, d % 8, :]
                nc.tensor.matmul(accu[:], wts["ws1T"][:, d, :], hs_d,
                                 start=(d == 0), stop=(d == ND - 1))
            nc.scalar.copy(u17[0:16, tsl], accu[:])

            # slot logits for this T-eighth (esT chunk + 2 es_t/cumsum chunks)
            pse = p1ps.tile([128, 256], F32, tag="acc_v0", name=f"psesT{tb}")
            nc.tensor.matmul(pse[:], c["ws2e"][:], u17[:, tsl], start=True, stop=True)
            tmpe = p1sb.tile([128, 256], F32, tag="sclamp")
            nc.vector.tensor_scalar_min(tmpe[:], pse[:], CLAMP)
            nc.vector.tensor_scalar_max(tmpe[:], tmpe[:], -CLAMP)
            nc.scalar.activation(esT[:, tsl], tmpe[:], AF.Exp)
            for ts in (2 * tb, 2 * tb + 1):
                pss = p1ps.tile([128, 128], F32, tag="acc_v1", name=f"ps_st{ts}")
                nc.tensor.matmul(pss[:], u17[:, ts * 128:(ts + 1) * 128],
                                 c["ws2e"][:], start=True, stop=True)
                tmp = p1sb.tile([128, 128], F32, tag="sclamp2")
                nc.vector.tensor_scalar_min(tmp[:], pss[:], CLAMP)
                nc.vector.tensor_scalar_max(tmp[:], tmp[:], -CLAMP)
                nc.scalar.activation(es_t[:, ts, :], tmp[:], AF.Exp)
                cs2 = p1ps.tile([128, 128], F32, tag="acc_wg0", name=f"ps_cs{ts}")
                nc.tensor.matmul(cs2[:], c["triu"][:], es_t[:, ts, :],
                                 start=True, stop=(ts == 0))
                if ts > 0:
                    nc.tensor.matmul(cs2[:], c["onesrow"][:], carries[ts - 1][:],
                                     start=False, stop=True)
                nc.vector.reciprocal(enz[:, ts, :], cs2[:])
                if ts < NT - 1:
                    csum = p1ps.tile([1, 128], F32, tag="acc_wg1",
                                     name=f"csum{ts}")
                    nc.tensor.matmul(csum[:], c["onescol"][:], es_t[:, ts, :],
                                     start=True, stop=(ts == 0))
                    if ts > 0:
                        nc.tensor.matmul(csum[:], c["onesrow"][:, 0:1],
                                         carries[ts - 1][:], start=False, stop=True)
                    cr = p1sb.tile([1, 128], F32R, tag="carry", name=f"carry{ts}",
                                   bufs=4)
                    nc.vector.tensor_copy(cr[:], csum[:])
                    carries.append(cr)
        p1ps_cm.__exit__(None, None, None)

    if debug:
        for h in range(2):
            nc.sync.dma_start(dbg["qT"].ap()[h * 128:(h + 1) * 128, :], qT[h][:].bitcast(F32))
            nc.sync.dma_start(dbg["kT"].ap()[h * 128:(h + 1) * 128, :], kT[h][:].bitcast(F32))
            nc.sync.dma_start(dbg["sg"].ap()[h * 128:(h + 1) * 128, :], sg[h][:])
        nc.sync.dma_start(dbg["v"].ap(), v[:].rearrange("p a b -> p (a b)").bitcast(F32))
        nc.sync.dma_start(dbg["es"].ap(), es_t[:].rearrange("p a b -> p (a b)").bitcast(F32))
        nc.sync.dma_start(dbg["enz"].ap(), enz[:].rearrange("p a b -> p (a b)"))
        nc.sync.dma_start(dbg["esT"].ap(), esT[:].bitcast(F32))

    # ================= PHASE 2: attention =================
    with tc.tile_pool(name="p2sb", bufs=3) as p2sb, \
         tc.tile_pool(name="qveP", bufs=2) as qvep, \
         tc.tile_pool(name="dramp", bufs=1, space="DRAM") as dpool:
        ag_in = dpool.tile([2, 128, T], BF16, tag="ag_in")
        ag_out = dpool.tile([N_CORES, 2, 128, T], BF16, tag="ag_out")
        epsb = p2sb.tile([1, 1], F32, tag="epsb")
        nc.vector.memset(epsb[:], EPS)
        for nm, dt in (("masks", F32), ("ident", F32)):
            tl = p2sb.tile(list(din[nm].shape), dt, tag=nm, name=nm, bufs=1)
            nc.sync.dma_start(tl[:], din[nm].ap())
            c[nm] = tl

        with tc.tile_pool(name="ps_at", bufs=3, space="PSUM") as ps_at, \
             tc.tile_pool(name="ps_ok", bufs=2, space="PSUM") as ps_ok, \
             tc.tile_pool(name="ps_ot", bufs=2, space="PSUM") as ps_ot, \
             tc.tile_pool(name="ps_aux", bufs=1, space="PSUM") as ps_aux:
            def stage1(I):
                rsl = slice(I * 512, (I + 1) * 512)
                njs = 4 * I + 4
                okT = [ps_ok.tile([64, 512], F32, tag=f"okT{i}", name=f"okT{i}",
                                  bufs=1) for i in range(2)]
                at_sb = {}

                def at_pair(j):
                    for h in range(2):
                        at = ps_at.tile([128, 512], F32, tag="atw2")
                        nc.tensor.matmul(at[:], kT[h][:, j * 128:(j + 1) * 128],
                                         qT[h][:, rsl], start=True, stop=True)
                        a_sb = p2sb.tile([128, 512], F32R, tag="at_sb")
                        m = j - 4 * I
                        if m >= 0:
                            nc.vector.tensor_mul(a_sb[:], at[:], c["masks"][:, m, :])
                        else:
                            nc.scalar.copy(a_sb[:], at[:])
                        at_sb[(j, h)] = a_sb

                at_pair(0)
                for j in range(njs):
                    if j + 1 < njs:
                        at_pair(j + 1)
                    for h in range(2):
                        nc.tensor.matmul(okT[h][:],
                                         es_t[:, j, h * 64:(h + 1) * 64],
                                         at_sb.pop((j, h))[:],
                                         start=(j == 0),
                                         stop=(j == njs - 1))
                return okT

            def softmax(I, okT):
                okT_sb = p2sb.tile([128, 512], F32, tag="okT_sb")
                nc.scalar.copy(okT_sb[0:64, :], okT[0][:])
                nc.scalar.copy(okT_sb[64:128, :], okT[1][:])
                qveT = qvep.tile([128, 512], F32R, tag="qveT")
                for r in range(4):
                    ts = 4 * I + r
                    okp = ps_aux.tile([128, 128], F32, tag="aux")
                    nc.tensor.transpose(okp[:], okT_sb[:, r * 128:(r + 1) * 128],
                                        c["ident"][:])
                    okm = p2sb.tile([128, 128], F32, tag="okm")
                    nc.vector.tensor_mul(okm[:], okp[:], enz[:, ts, :])
                    if debug:
                        nc.sync.dma_start(dbg["ok"].ap()[:, ts * 128:(ts + 1) * 128], okm[:])
                    eok = p2sb.tile([128, 128], F32, tag="eok")
                    nc.scalar.activation(eok[:], okm[:], AF.Exp)
                    ssum = p2sb.tile([128, 2], F32, tag="ssum")
                    nc.vector.reduce_sum(ssum[:], eok[:].rearrange("p (g m) -> p g m", g=2),
                                         axis=mybir.AxisListType.X)
                    rcp = p2sb.tile([128, 2], F32, tag="rcp")
                    nc.vector.reciprocal(rcp[:], ssum[:])
                    qve = p2sb.tile([128, 128], F32, tag="qve")
                    for h in range(2):
                        hs_ = slice(h * 64, (h + 1) * 64)
                        nc.vector.tensor_scalar_mul(qve[:, hs_], eok[:, hs_], rcp[:, h:h + 1])
                        nc.vector.tensor_mul(qve[:, hs_], qve[:, hs_], enz[:, ts, hs_])
                    qvp = ps_aux.tile([128, 128], F32, tag="aux")
                    nc.tensor.transpose(qvp[:], qve[:], c["ident"][:])
                    nc.vector.tensor_copy(qveT[:, r * 128:(r + 1) * 128], qvp[:])
                return qveT

            def stage2(I, qveT):
                rsl = slice(I * 512, (I + 1) * 512)
                njs = 4 * I + 4
                oT = [ps_ot.tile([128, 512], F32, tag="oT", name=f"oT{i}") for i in range(2)]
                w2_sb = {}

                def w2_pair(j):
                    for h in range(2):
                        w2 = ps_at.tile([128, 512], F32, tag="atw2")
                        nc.tensor.matmul(w2[:],
                                         esT[h * 64:(h + 1) * 64, j * 128:(j + 1) * 128],
                                         qveT[h * 64:(h + 1) * 64, :],
                                         start=True, stop=True)
                        wsb = p2sb.tile([128, 512], F32R, tag="at_sb")
                        m = j - 4 * I
                        if m >= 0:
                            nc.vector.tensor_mul(wsb[:], w2[:], c["masks"][:, m, :])
                        else:
                            nc.scalar.copy(wsb[:], w2[:])
                        w2_sb[(j, h)] = wsb

                w2_pair(0)
                for j in range(njs):
                    if j + 1 < njs:
                        w2_pair(j + 1)
                    for h in range(2):
                        nc.tensor.matmul(oT[h][:], v[:, j, h * 128:(h + 1) * 128],
                                         w2_sb.pop((j, h))[:],
                                         start=(j == 0), stop=(j == njs - 1))
                # epilogue: o_g = o * rms(o) * sg  -> bf16 -> a2a_in
                for h in range(2):
                    sq = p2sb.tile([128, 512], F32R, tag="sq")
                    nc.scalar.activation(sq[:], oT[h][:], AF.Square)
                    ssq = ps_aux.tile([1, 512], F32, tag="aux")
                    nc.tensor.matmul(ssq[:], c["onescol"][:], sq[:], start=True, stop=True)
                    rms = p2sb.tile([1, 512], F32, tag="rms")
                    nc.scalar.activation(rms[:], ssq[:], AF.Sqrt, scale=1.0 / DV, bias=epsb[:])
                    rinv = p2sb.tile([1, 512], F32R, tag="rinv")
                    with nc.allow_low_precision(reason="rinv f32r for matmul broadcast"):
                        nc.vector.reciprocal(rinv[:], rms[:])
                    rb = ps_aux.tile([128, 512], F32, tag="aux")
                    nc.tensor.matmul(rb[:], c["onesrow"][:], rinv[:], start=True, stop=True)
                    t1 = p2sb.tile([128, 512], F32, tag="ept1")
                    nc.vector.tensor_mul(t1[:], oT[h][:], sg[h][:, rsl])
                    ogt = p2sb.tile([128, 512], BF16, tag="ogt")
                    nc.vector.tensor_mul(ogt[:], t1[:], rb[:])
                    if debug:
                        nc.gpsimd.dma_start(dbg["ogT"].ap()[h * 128:(h + 1) * 128, rsl], ogt[:])
                    nc.sync.dma_start(ag_in[h, :, rsl], ogt[:])

            ok_prev = stage1(0)
            qv_prev = softmax(0, ok_prev)
            if debug:
                nc.sync.dma_start(dbg["qveT"].ap()[:, 0:512], qv_prev[:].bitcast(F32))
            for I in range(1, NB):
                okI = stage1(I)
                stage2(I - 1, qv_prev)
                qv_prev = softmax(I, okI)
                if debug:
                    nc.sync.dma_start(dbg["qveT"].ap()[:, I * 512:(I + 1) * 512], qv_prev[:].bitcast(F32))
            stage2(NB - 1, qv_prev)

        # ================= PHASE 3: A2A + o_proj =================
        if nocc:
            nc.sync.dma_start(ag_out[0], ag_in[:])
        else:
            nc.gpsimd.collective_compute(
                "AllGather", mybir.AluOpType.bypass,
                replica_groups=[list(range(N_CORES))],
                ins=[ag_in[:].opt()], outs=[ag_out[:].opt()])

        import concourse.bass as bass_mod
        pid = nc.sync.partition_id()
        toff = pid * 256
        og_sb = p2sb.tile([128, 16, 256], BF16, tag="og_sb")
        nc.sync.dma_start(
            og_sb[:],
            ag_out[:].rearrange("s h p t -> p (s h) t")[:, :, bass_mod.ds(toff, 256)])
        wo_d = din["woT"].ap().rearrange("(k p) t -> p k t", p=128)
        p3ps_cm = tc.tile_pool(name="p3ps", bufs=1, space="PSUM")
        p3ps = p3ps_cm.__enter__()
        pso = {}
        for th in range(2):
            for ns in range(4):
                pso[(th, ns)] = p3ps.tile([128, 512], F32, tag=f"pso{th}{ns}",
                                          name=f"pso{th}{ns}", bufs=1)
        for kc in range(16):
            woc = p2sb.tile([128, D], BF16, tag="woc", bufs=3, name=f"woc{kc}")
            nc.sync.dma_start(woc[:], wo_d[:, kc, :])
            for th in range(2):
                for ns in range(4):
                    nc.tensor.matmul(pso[(th, ns)][:],
                                     og_sb[:, kc, th * 128:(th + 1) * 128],
                                     woc[:, ns * 512:(ns + 1) * 512],
                                     start=(kc == 0), stop=(kc == 15))
        for th in range(2):
            for ns in range(4):
                osb = p2sb.tile([128, 512], F32, tag="osb")
                nc.scalar.copy(osb[:], pso[(th, ns)][:])
                nc.sync.dma_start(
                    out_d.ap()[th * 128:(th + 1) * 128, ns * 512:(ns + 1) * 512],
                    osb[:])
        p3ps_cm.__exit__(None, None, None)


# ======================= host side =======================

def _host_inputs(inputs):
    import ml_dtypes
    hs = np.ascontiguousarray(np.asarray(inputs["hidden_states"], np.float32)[0])
    Wq = np.asarray(inputs["Wq"], np.float32)
    Wk = np.asarray(inputs["Wk"], np.float32)
    Wv = np.asarray(inputs["Wv"], np.float32)
    Wg = np.asarray(inputs["Wg"], np.float32)
    Wo = np.asarray(inputs["Wo"], np.float32)
    Ws1 = np.asarray(inputs["Ws1"], np.float32)
    Ws2 = np.asarray(inputs["Ws2"], np.float32)
    bs2 = np.asarray(inputs["bs2"], np.float32)
    gnw = np.asarray(inputs["g_norm_weight"], np.float32)

    hsT = np.ascontiguousarray(hs.T)
    pos = np.arange(T, dtype=np.float64)
    inv = 1.0 / (ROPE_BASE ** (np.arange(0, DK, 2, dtype=np.float64) / DK))
    ang = pos[:, None] * inv[None, :]
    cos = np.cos(ang).T.astype(np.float32)       # [64, T]
    sin = np.sin(ang).T.astype(np.float32)
    cossin = np.concatenate([cos, sin], axis=0).astype(np.float32)
    triu = np.triu(np.ones((128, 128), np.float32))
    masks = np.zeros((128, 4, 512), np.float32)
    p = np.arange(128)[:, None]
    r = np.arange(512)[None, :]
    for m in range(4):
        masks[:, m, :] = (128 * m + p <= r).astype(np.float32)
    ident = np.eye(128, dtype=np.float32)
    onesrow = np.ones((1, 128), np.float32)
    ones2k = np.ones((1, T), np.float32)
    onescol = np.ones((128, 1), np.float32)
    woT = (Wo.T * np.tile(gnw, H)[:, None]).astype(ml_dtypes.bfloat16)

    in_maps = []
    for core in range(N_CORES):
        sl = slice(core * 256, (core + 1) * 256)
        ssl = slice(core * 128, (core + 1) * 128)
        ws2e = np.concatenate([Ws2[ssl].T, bs2[None, ssl]], axis=0).astype(np.float32)
        m = {
            "hsT": hsT,
            "wq0": np.ascontiguousarray(Wq[sl].T[:, 0:128] * SCALE),
            "wq1": np.ascontiguousarray(Wq[sl].T[:, 128:256] * SCALE),
            "wk0": np.ascontiguousarray(Wk[sl].T[:, 0:128]),
            "wk1": np.ascontiguousarray(Wk[sl].T[:, 128:256]),
            "wg0": np.ascontiguousarray(Wg[sl].T[:, 0:128]),
            "wg1": np.ascontiguousarray(Wg[sl].T[:, 128:256]),
            "wvT": np.ascontiguousarray(Wv[sl].T),
            "ws1T": np.ascontiguousarray(Ws1.T),
            "ws2e": ws2e,
            "onesrow": onesrow, "onescol": onescol, "ones2k": ones2k,
            "cossin": cossin,
            "triu": triu, "masks": masks, "ident": ident,
            "woT": woT,
        }
        in_maps.append(m)
    return in_maps


_CACHE = {}


def kernel(**inputs):
    key = ("k", REPEAT, DEBUG)
    if key not in _CACHE:
        _CACHE[key] = build(repeat=REPEAT, debug=DEBUG)
    nc, dbg = _CACHE[key]
    in_maps = _host_inputs(inputs)
    res = bass_utils.run_bass_kernel_spmd(nc, in_maps, core_ids=list(range(N_CORES)))
    out = np.concatenate([res.results[c]["out"] for c in range(N_CORES)], axis=0)
    kernel.last_results = res
    return out.reshape(1, T, D).astype(np.float32)



# revision 6
# speedup vs baseline: 1.3859x; 1.3859x over previous
"""ABC attention (gated slot attention) on 8 TRN2 NeuronCores.

Sharding: 2 heads per core (16 heads / 8 cores). Per core:
  - projections q,k (RoPE, q pre-scaled), v, silu(gate), slot logits,
    all matmuls bf16 (2x stream rate vs f32r), moving dim 512
  - unnormalized softmax: RMS-norm downstream is scale-invariant, so
    softmax keeps only exp(ok*enz)*enz; enz applied in [m,t] layout
    (enzT) -> no per-row transposes/reductions/reciprocals
  - quadratic chunked ABC attention, causal masking via mask tiles
  - fused RMS-norm x gate epilogue (Rsqrt broadcast via PE)
  - AllToAll reshards o_g head-split -> T-split (1MB/core vs 8.4MB
    AllGather); per-core o_proj over its 256-row T slice.
"""
import sys
if '/opt/trn_rl_repo' not in sys.path:
    sys.path.insert(0, '/opt/trn_rl_repo')
import numpy as np

import concourse.bacc as bacc
import concourse.mybir as mybir
import concourse.tile as tile
from concourse import bass_utils

F32 = mybir.dt.float32
F32R = mybir.dt.float32r
BF16 = mybir.dt.bfloat16
AF = mybir.ActivationFunctionType

H, DK, DV, M, T, D = 16, 128, 128, 64, 2048, 2048
EPS, CLAMP, ROPE_BASE = 1e-5, 32.0, 10000.0
N_CORES = 8
NT = T // 128        # 16
NB = T // 512        # 4 big row-chunks
ND = D // 128        # 16
SCALE = DK ** -0.5

REPEAT = 1
DEBUG = False


def build(repeat=1, debug=False):
    nc = bacc.Bacc(None, target_bir_lowering=False, debug=False, num_devices=N_CORES)

    din = {}
    for nm, shp, dt in [
        ("hsb", [128, NB, ND, 512], BF16),
        ("wq0", [128, ND, 128], BF16), ("wq1", [128, ND, 128], BF16),
        ("wk0", [128, ND, 128], BF16), ("wk1", [128, ND, 128], BF16),
        ("wg0", [128, ND, 128], BF16), ("wg1", [128, ND, 128], BF16),
        ("wvu", [128, ND, 272], BF16),
        ("ws2e", [17, 128], BF16),
        ("ones2k", [1, T], BF16),
        ("onesrow_b", [1, 128], BF16), ("onescol_b", [128, 1], BF16),
        ("onesrow_r", [1, 128], F32R), ("onescol_r", [128, 1], F32R),
        ("cossin", [128, T], F32),
        ("triu", [128, 128], BF16), ("ident", [128, 128], F32),
        ("masks", [128, 4, 512], F32),
        ("woT", [128, ND, D], BF16),
    ]:
        din[nm] = nc.dram_tensor(nm, shp, dt, kind="ExternalInput")
    out_d = nc.dram_tensor("out", [256, D], F32, kind="ExternalOutput")

    dbg = {}
    if debug:
        for nm, shp, dt in [("qT", [256, T], BF16), ("kT", [256, T], BF16),
                            ("v", [128, NT * 256], BF16),
                            ("sg", [256, T], BF16), ("es", [128, NT * 128], BF16),
                            ("enzT", [128, T], F32), ("esT", [128, T], BF16),
                            ("u17", [17, T], BF16), ("qveT", [128, T], BF16),
                            ("ogT", [256, T], BF16)]:
            dbg[nm] = nc.dram_tensor("dbg_" + nm, shp, dt, kind="ExternalOutput")

    with tile.TileContext(nc) as tc:
        with tc.tile_pool(name="const", bufs=1) as cpool, \
             tc.tile_pool(name="big", bufs=1) as big:
            c = {}
            for nm in ("ws2e", "onesrow_b", "onescol_b", "onesrow_r",
                       "onescol_r", "triu", "ident"):
                tl = cpool.tile(list(din[nm].shape), din[nm].dtype, tag=nm, name=nm)
                nc.sync.dma_start(tl[:], din[nm].ap())
                c[nm] = tl

            st = {
                "qT": [big.tile([128, T], BF16, tag=f"qT{h}", name=f"qT{h}") for h in range(2)],
                "kT": [big.tile([128, T], BF16, tag=f"kT{h}", name=f"kT{h}") for h in range(2)],
                "sg": [big.tile([128, T], BF16, tag=f"sg{h}", name=f"sg{h}") for h in range(2)],
                "v": big.tile([128, NT, 256], BF16, tag="v", name="v"),
                "u17": big.tile([17, T], BF16, tag="u17", name="u17"),
                "es_t": big.tile([128, NT, 128], BF16, tag="es_t", name="es_t"),
                "esT": big.tile([128, T], BF16, tag="esT", name="esT"),
                "enzT": big.tile([128, T], F32, tag="enzT", name="enzT"),
            }
            for _ in range(repeat):
                _pass(nc, tc, din, c, st, out_d, dbg, debug)

    nc.compile()
    return nc, dbg


def _pass(nc, tc, din, c, st, out_d, dbg, debug):
    qT, kT, sg = st["qT"], st["kT"], st["sg"]
    v, u17, es_t, esT, enzT = st["v"], st["u17"], st["es_t"], st["esT"], st["enzT"]

    # ================= PHASE 1: projections (4 x 512-col sweeps) =================
    with tc.tile_pool(name="p1w", bufs=1) as p1w, \
         tc.tile_pool(name="p1sb", bufs=2) as p1sb, \
         tc.tile_pool(name="p1hs", bufs=3) as p1hs:
        p1ps_cm = tc.tile_pool(name="p1ps", bufs=1, space="PSUM")
        p1ps = p1ps_cm.__enter__()
        cossin = p1w.tile([128, T], F32, tag="cossin", name="cossin")
        nc.sync.dma_start(cossin[:], din["cossin"].ap())
        wts = {}
        for wn in ("wq0", "wq1", "wk0", "wk1", "wg0", "wg1", "wvu"):
            cw = din[wn].shape[2]
            wt = p1w.tile([128, ND, cw], BF16, tag=wn, name=wn)
            nc.sync.dma_start(wt[:], din[wn].ap())
            wts[wn] = wt
        nc.sync.dma_start(u17[16:17, :], din["ones2k"].ap())

        carries = []
        QKG = (("wq0", qT[0], "q"), ("wq1", qT[1], "q"),
               ("wk0", kT[0], "k"), ("wk1", kT[1], "k"),
               ("wg0", sg[0], "g"), ("wg1", sg[1], "g"))
        for tb in range(NB):
            tsl = slice(tb * 512, (tb + 1) * 512)
            hsq = p1hs.tile([128, ND, 512], BF16, tag="hsq", name=f"hsq{tb}")
            nc.sync.dma_start(hsq[:], din["hsb"].ap()[:, tb])
            acc = {wn: p1ps.tile([128, 512], F32, tag=f"acc_{wn}",
                                 name=f"acc_{wn}_{tb}")
                   for wn, *_ in QKG}
            accv = [p1ps.tile([128, 272], F32, tag=f"acc_v{i}", name=f"acc_v{i}_{tb}")
                    for i in range(2)]
            for d in range(ND):
                hs_d = hsq[:, d, :]
                for wn, dst, kind in QKG:
                    nc.tensor.matmul(acc[wn][:], wts[wn][:, d, :], hs_d,
                                     start=(d == 0), stop=(d == ND - 1))
                for i in range(2):
                    nc.tensor.matmul(accv[i][:],
                                     hs_d[:, i * 128:(i + 1) * 128],
                                     wts["wvu"][:, d, :],
                                     start=(d == 0), stop=(d == ND - 1))
            # drains: RoPE for q/k, silu for g
            for wn, dst, kind in QKG:
                ps = acc[wn]
                if kind == "g":
                    sgm = p1sb.tile([128, 512], F32, tag="sgm")
                    nc.scalar.activation(sgm[:], ps[:], AF.Sigmoid)
                    nc.vector.tensor_mul(dst[:, tsl], ps[:], sgm[:])
                else:
                    t1 = p1sb.tile([64, 512], F32, tag="ropet1")
                    t2 = p1sb.tile([64, 512], F32, tag="ropet2")
                    nc.vector.tensor_mul(t1[:], ps[0:64, :], cossin[0:64, tsl])
                    nc.vector.tensor_mul(t2[:], ps[64:128, :], cossin[64:128, tsl])
                    nc.vector.tensor_sub(dst[0:64, tsl], t1[:], t2[:])
                    nc.vector.tensor_mul(t1[:], ps[64:128, :], cossin[0:64, tsl])
                    nc.vector.tensor_mul(t2[:], ps[0:64, :], cossin[64:128, tsl])
                    nc.vector.tensor_add(dst[64:128, tsl], t1[:], t2[:])
            # v+u drains for subblocks 0,1; then second v+u pass for 2,3
            def drain_vu(i, accv_i):
                ts = 4 * tb + i
                nc.scalar.copy(v[:, ts, :], accv_i[:, 0:256])
                usb = p1sb.tile([128, 16], F32, tag="usb")
                nc.scalar.copy(usb[:], accv_i[:, 256:272])
                utp = p1ps.tile([16, 128], F32, tag="acc_wq0", name=f"utp{ts}")
                nc.tensor.transpose(utp[:], usb[:], c["ident"][:])
                nc.scalar.copy(u17[0:16, ts * 128:(ts + 1) * 128], utp[:])

            drain_vu(0, accv[0])
            drain_vu(1, accv[1])
            accv2 = [p1ps.tile([128, 272], F32, tag=f"acc_v{i}", name=f"acc_v2{i}_{tb}")
                     for i in range(2)]
            for d in range(ND):
                hs_d = hsq[:, d, :]
                for i in range(2):
                    nc.tensor.matmul(accv2[i][:],
                                     hs_d[:, (2 + i) * 128:(3 + i) * 128],
                                     wts["wvu"][:, d, :],
                                     start=(d == 0), stop=(d == ND - 1))
            drain_vu(2, accv2[0])
            drain_vu(3, accv2[1])

            # slot logits for this chunk: esT (col-oriented) + es_t/cumsum
            pse = p1ps.tile([128, 512], F32, tag="acc_wk0", name=f"psesT{tb}")
            nc.tensor.matmul(pse[:], c["ws2e"][:], u17[:, tsl], start=True, stop=True)
            nc.scalar.activation(esT[:, tsl], pse[:], AF.Exp)
            for ts in range(4 * tb, 4 * tb + 4):
                ssl = slice(ts * 128, (ts + 1) * 128)
                pss = p1ps.tile([128, 128], F32, tag="acc_wk1", name=f"ps_st{ts}")
                nc.tensor.matmul(pss[:], u17[:, ssl], c["ws2e"][:],
                                 start=True, stop=True)
                nc.scalar.activation(es_t[:, ts, :], pss[:], AF.Exp)
                cs2 = p1ps.tile([128, 128], F32, tag="acc_wg0", name=f"ps_cs{ts}")
                nc.tensor.matmul(cs2[:], c["triu"][:], es_t[:, ts, :],
                                 start=True, stop=(ts == 0))
                if ts > 0:
                    nc.tensor.matmul(cs2[:], c["onesrow_b"][:], carries[ts - 1][:],
                                     start=False, stop=True)
                enz_sb = p1sb.tile([128, 128], F32, tag="enz_sb")
                nc.vector.reciprocal_approx_fast(enz_sb[:], cs2[:])
                etp = p1ps.tile([128, 128], F32, tag="acc_wg1", name=f"etp{ts}")
                nc.tensor.transpose(etp[:], enz_sb[:], c["ident"][:])
                nc.scalar.copy(enzT[:, ssl], etp[:])
                if ts < NT - 1:
                    csum = p1ps.tile([1, 128], F32, tag="acc_wq1",
                                     name=f"csum{ts}")
                    nc.tensor.matmul(csum[:], c["onescol_b"][:], es_t[:, ts, :],
                                     start=True, stop=(ts == 0))
                    if ts > 0:
                        nc.tensor.matmul(csum[:], c["onesrow_b"][:, 0:1],
                                         carries[ts - 1][:], start=False, stop=True)
                    cr = p1sb.tile([1, 128], BF16, tag="carry", name=f"carry{ts}",
                                   bufs=4)
                    nc.vector.tensor_copy(cr[:], csum[:])
                    carries.append(cr)
        p1ps_cm.__exit__(None, None, None)

    if debug:
        for h in range(2):
            nc.sync.dma_start(dbg["qT"].ap()[h * 128:(h + 1) * 128, :], qT[h][:])
            nc.sync.dma_start(dbg["kT"].ap()[h * 128:(h + 1) * 128, :], kT[h][:])
            nc.sync.dma_start(dbg["sg"].ap()[h * 128:(h + 1) * 128, :], sg[h][:])
        nc.sync.dma_start(dbg["v"].ap(), v[:].rearrange("p a b -> p (a b)"))
        nc.sync.dma_start(dbg["es"].ap(), es_t[:].rearrange("p a b -> p (a b)"))
        nc.sync.dma_start(dbg["enzT"].ap(), enzT[:])
        nc.sync.dma_start(dbg["esT"].ap(), esT[:])
        nc.sync.dma_start(dbg["u17"].ap(), u17[:])

    # ================= PHASE 2: attention =================
    with tc.tile_pool(name="p2sb", bufs=3) as p2sb, \
         tc.tile_pool(name="qveP", bufs=2) as qvep, \
         tc.tile_pool(name="dramp", bufs=1, space="DRAM") as dpool:
        ag_in = dpool.tile([N_CORES, 2, 128, 256], BF16, tag="ag_in")
        ag_out = dpool.tile([N_CORES, 2, 128, 256], BF16, tag="ag_out")
        epsb = p2sb.tile([128, 1], F32, tag="epsb", bufs=1)
        nc.vector.memset(epsb[:], EPS)
        tl = p2sb.tile(list(din["masks"].shape), F32, tag="masks", name="masks", bufs=1)
        nc.sync.dma_start(tl[:], din["masks"].ap())
        c["masks"] = tl

        with tc.tile_pool(name="ps_at", bufs=3, space="PSUM") as ps_at, \
             tc.tile_pool(name="ps_ok", bufs=2, space="PSUM") as ps_ok, \
             tc.tile_pool(name="ps_ot", bufs=2, space="PSUM") as ps_ot, \
             tc.tile_pool(name="ps_aux", bufs=1, space="PSUM") as ps_aux:
            def stage1(I):
                rsl = slice(I * 512, (I + 1) * 512)
                njs = 4 * I + 4
                okT = ps_ok.tile([128, 512], F32, tag="okT", name=f"okT{I}",
                                 bufs=2)
                at_sb = {}

                def at_pair(j):
                    for h in range(2):
                        at = ps_at.tile([128, 512], F32, tag="atw2")
                        nc.tensor.matmul(at[:], kT[h][:, j * 128:(j + 1) * 128],
                                         qT[h][:, rsl], start=True, stop=True)
                        a_sb = p2sb.tile([128, 512], BF16, tag="at_sb")
                        m = j - 4 * I
                        if m >= 0:
                            nc.vector.tensor_mul(a_sb[:], at[:], c["masks"][:, m, :])
                        else:
                            nc.scalar.copy(a_sb[:], at[:])
                        at_sb[(j, h)] = a_sb

                at_pair(0)
                for j in range(njs):
                    if j + 1 < njs:
                        at_pair(j + 1)
                    for h in range(2):
                        nc.tensor.matmul(okT[h * 64:(h + 1) * 64, :],
                                         es_t[:, j, h * 64:(h + 1) * 64],
                                         at_sb.pop((j, h))[:],
                                         start=(j == 0),
                                         stop=(j == njs - 1))
                return okT

            def softmax(I, okT):
                # unnormalized: qveT = exp(okT * enzT) * enzT  (RMS-norm
                # downstream cancels the per-(t,h) softmax denominator)
                rsl = slice(I * 512, (I + 1) * 512)
                qveT = qvep.tile([128, 512], BF16, tag="qveT")
                okm = p2sb.tile([128, 512], F32, tag="okm")
                nc.vector.tensor_mul(okm[:], okT[:], enzT[:, rsl])
                eok = p2sb.tile([128, 512], F32, tag="eok")
                nc.scalar.activation(eok[:], okm[:], AF.Exp)
                nc.vector.tensor_mul(qveT[:], eok[:], enzT[:, rsl])
                return qveT

            def stage2(I, qveT):
                rsl = slice(I * 512, (I + 1) * 512)
                njs = 4 * I + 4
                oT = [ps_ot.tile([128, 512], F32, tag="oT", name=f"oT{i}") for i in range(2)]
                w2_sb = {}

                def w2_pair(j):
                    for h in range(2):
                        w2 = ps_at.tile([128, 512], F32, tag="atw2")
                        nc.tensor.matmul(w2[:],
                                         esT[h * 64:(h + 1) * 64, j * 128:(j + 1) * 128],
                                         qveT[h * 64:(h + 1) * 64, :],
                                         start=True, stop=True)
                        wsb = p2sb.tile([128, 512], BF16, tag="at_sb")
                        m = j - 4 * I
                        if m >= 0:
                            nc.vector.tensor_mul(wsb[:], w2[:], c["masks"][:, m, :])
                        else:
                            nc.scalar.copy(wsb[:], w2[:])
                        w2_sb[(j, h)] = wsb

                w2_pair(0)
                for j in range(njs):
                    if j + 1 < njs:
                        w2_pair(j + 1)
                    for h in range(2):
                        nc.tensor.matmul(oT[h][:], v[:, j, h * 128:(h + 1) * 128],
                                         w2_sb.pop((j, h))[:],
                                         start=(j == 0), stop=(j == njs - 1))
                # epilogue: o_g = o * rsqrt(mean o^2 + eps) * sg -> bf16 -> a2a_in
                for h in range(2):
                    sq = p2sb.tile([128, 512], F32R, tag="sq")
                    nc.scalar.activation(sq[:], oT[h][:], AF.Square)
                    ssq = ps_aux.tile([1, 512], F32, tag="aux")
                    nc.tensor.matmul(ssq[:], c["onescol_r"][:], sq[:], start=True, stop=True)
                    ssq_sb = p2sb.tile([1, 512], F32R, tag="ssq_sb")
                    with nc.allow_low_precision(reason="f32r bitcast for broadcast"):
                        nc.scalar.copy(ssq_sb[:], ssq[:])
                    rb = ps_aux.tile([128, 512], F32, tag="aux")
                    nc.tensor.matmul(rb[:], c["onesrow_r"][:], ssq_sb[:], start=True, stop=True)
                    rms = p2sb.tile([128, 512], F32, tag="rms")
                    nc.scalar.activation(rms[:], rb[:], AF.Sqrt, scale=1.0 / DV,
                                         bias=epsb[:])
                    rinv = p2sb.tile([128, 512], F32, tag="rinv")
                    nc.vector.reciprocal_approx_fast(rinv[:], rms[:])
                    t1 = p2sb.tile([128, 512], F32, tag="ept1")
                    nc.vector.tensor_mul(t1[:], oT[h][:], sg[h][:, rsl])
                    ogt = p2sb.tile([128, 512], BF16, tag="ogt")
                    nc.vector.tensor_mul(ogt[:], t1[:], rinv[:])
                    if debug:
                        nc.gpsimd.dma_start(dbg["ogT"].ap()[h * 128:(h + 1) * 128, rsl], ogt[:])
                    nc.sync.dma_start(ag_in[2 * I, h, :, :], ogt[:, 0:256])
                    nc.sync.dma_start(ag_in[2 * I + 1, h, :, :], ogt[:, 256:512])

            ok_prev = stage1(0)
            qv_prev = softmax(0, ok_prev)
            if debug:
                nc.sync.dma_start(dbg["qveT"].ap()[:, 0:512], qv_prev[:])
            for I in range(1, NB):
                okI = stage1(I)
                stage2(I - 1, qv_prev)
                qv_prev = softmax(I, okI)
                if debug:
                    nc.sync.dma_start(dbg["qveT"].ap()[:, I * 512:(I + 1) * 512], qv_prev[:])
            stage2(NB - 1, qv_prev)

        # ================= PHASE 3: A2A + o_proj =================
        nc.gpsimd.collective_compute(
            "AllToAll", mybir.AluOpType.bypass,
            replica_groups=[list(range(N_CORES))],
            ins=[ag_in[:].opt()], outs=[ag_out[:].opt()])

        # og_sb[p, kc, t]: kc = (src core, head); per-kc tiles so o_proj
        # matmuls start as soon as each chunk lands
        og = []
        agv = ag_out[:].rearrange("s h p t -> p (s h) t")
        for kc in range(16):
            ot = p2sb.tile([128, 256], BF16, tag=f"og{kc}", name=f"og{kc}", bufs=1)
            nc.sync.dma_start(ot[:], agv[:, kc, :])
            og.append(ot)
        p3ps_cm = tc.tile_pool(name="p3ps", bufs=1, space="PSUM")
        p3ps = p3ps_cm.__enter__()
        pso = {}
        for th in range(2):
            for ns in range(4):
                pso[(th, ns)] = p3ps.tile([128, 512], F32, tag=f"pso{th}{ns}",
                                          name=f"pso{th}{ns}", bufs=1)
        for kc in range(16):
            woc = p2sb.tile([128, D], BF16, tag="woc", bufs=3, name=f"woc{kc}")
            nc.sync.dma_start(woc[:], din["woT"].ap()[:, kc, :])
            for th in range(2):
                for ns in range(4):
                    nc.tensor.matmul(pso[(th, ns)][:],
                                     og[kc][:, th * 128:(th + 1) * 128],
                                     woc[:, ns * 512:(ns + 1) * 512],
                                     start=(kc == 0), stop=(kc == 15))
        for th in range(2):
            for ns in range(4):
                osb = p2sb.tile([128, 512], F32, tag="osb")
                nc.scalar.copy(osb[:], pso[(th, ns)][:])
                nc.sync.dma_start(
                    out_d.ap()[th * 128:(th + 1) * 128, ns * 512:(ns + 1) * 512],
                    osb[:])
        p3ps_cm.__exit__(None, None, None)


# ======================= host side =======================

def _host_inputs(inputs):
    import ml_dtypes
    BF = ml_dtypes.bfloat16
    hs = np.ascontiguousarray(np.asarray(inputs["hidden_states"], np.float32)[0])
    Wq = np.asarray(inputs["Wq"], np.float32)
    Wk = np.asarray(inputs["Wk"], np.float32)
    Wv = np.asarray(inputs["Wv"], np.float32)
    Wg = np.asarray(inputs["Wg"], np.float32)
    Wo = np.asarray(inputs["Wo"], np.float32)
    Ws1 = np.asarray(inputs["Ws1"], np.float32)
    Ws2 = np.asarray(inputs["Ws2"], np.float32)
    bs2 = np.asarray(inputs["bs2"], np.float32)
    gnw = np.asarray(inputs["g_norm_weight"], np.float32)

    hsT = hs.T  # [D, T]
    # hsb: [p, chunk, k, t] with d = k*128 + p
    hsb = np.ascontiguousarray(
        hsT.reshape(ND, 128, NB, 512).transpose(1, 2, 0, 3)).astype(BF)
    pos = np.arange(T, dtype=np.float64)
    inv = 1.0 / (ROPE_BASE ** (np.arange(0, DK, 2, dtype=np.float64) / DK))
    ang = pos[:, None] * inv[None, :]
    cos = np.cos(ang).T.astype(np.float32)       # [64, T]
    sin = np.sin(ang).T.astype(np.float32)
    cossin = np.concatenate([cos, sin], axis=0).astype(np.float32)
    triu = np.triu(np.ones((128, 128), np.float32)).astype(BF)
    masks = np.zeros((128, 4, 512), np.float32)
    p = np.arange(128)[:, None]
    r = np.arange(512)[None, :]
    for m in range(4):
        masks[:, m, :] = (128 * m + p <= r).astype(np.float32)
    ident = np.eye(128, dtype=np.float32)
    onesrow = np.ones((1, 128), np.float32)
    ones2k = np.ones((1, T), np.float32).astype(BF)
    onescol = np.ones((128, 1), np.float32)
    # woT: [p, kc, n] with hd = kc*128 + p; gnw folded in
    woT = (Wo.T * np.tile(gnw, H)[:, None]).astype(BF)
    woT = np.ascontiguousarray(woT.reshape(ND, 128, D).transpose(1, 0, 2))

    def wlay(w):  # [2048, 128] -> [p, k, c] bf16
        return np.ascontiguousarray(
            w.reshape(ND, 128, -1).transpose(1, 0, 2)).astype(BF)

    in_maps = []
    for core in range(N_CORES):
        sl = slice(core * 256, (core + 1) * 256)
        ssl = slice(core * 128, (core + 1) * 128)
        ws2e = np.concatenate([Ws2[ssl].T, bs2[None, ssl]], axis=0).astype(BF)
        wvu = np.concatenate([Wv[sl].T, Ws1.T], axis=1)  # [2048, 272]
        m = {
            "hsb": hsb,
            "wq0": wlay(Wq[sl].T[:, 0:128] * SCALE),
            "wq1": wlay(Wq[sl].T[:, 128:256] * SCALE),
            "wk0": wlay(Wk[sl].T[:, 0:128]),
            "wk1": wlay(Wk[sl].T[:, 128:256]),
            "wg0": wlay(Wg[sl].T[:, 0:128]),
            "wg1": wlay(Wg[sl].T[:, 128:256]),
            "wvu": wlay(wvu),
            "ws2e": ws2e,
            "onesrow_b": onesrow.astype(BF), "onescol_b": onescol.astype(BF),
            "onesrow_r": onesrow, "onescol_r": onescol,
            "ones2k": ones2k,
            "cossin": cossin,
            "triu": triu, "masks": masks, "ident": ident,
            "woT": woT,
        }
        in_maps.append(m)
    return in_maps


_CACHE = {}


def kernel(**inputs):
    key = ("k", REPEAT, DEBUG)
    if key not in _CACHE:
        _CACHE[key] = build(repeat=REPEAT, debug=DEBUG)
    nc, dbg = _CACHE[key]
    in_maps = _host_inputs(inputs)
    res = bass_utils.run_bass_kernel_spmd(nc, in_maps, core_ids=list(range(N_CORES)))
    out = np.concatenate([res.results[c]["out"] for c in range(N_CORES)], axis=0)
    kernel.last_results = res
    return out.reshape(1, T, D).astype(np.float32)


# revision 16
# speedup vs baseline: 1.4676x; 1.0590x over previous
"""ABC attention (gated slot attention) on 8 TRN2 NeuronCores.

Sharding: 2 heads per core (16 heads / 8 cores). Per core:
  - projections q,k (RoPE, q pre-scaled), v, silu(gate), slot logits,
    all matmuls bf16 (2x stream rate vs f32r), moving dim 512
  - unnormalized softmax: RMS-norm downstream is scale-invariant, so
    softmax keeps only exp(ok*enz)*enz; enz applied in [m,t] layout
    (enzT) -> no per-row transposes/reductions/reciprocals
  - quadratic chunked ABC attention, causal masking via mask tiles
  - fused RMS-norm x gate epilogue (Rsqrt broadcast via PE)
  - AllToAll reshards o_g head-split -> T-split (1MB/core vs 8.4MB
    AllGather); per-core o_proj over its 256-row T slice.
"""
import sys
if '/opt/trn_rl_repo' not in sys.path:
    sys.path.insert(0, '/opt/trn_rl_repo')
import numpy as np

import concourse.bacc as bacc
import concourse.mybir as mybir
import concourse.tile as tile
from concourse import bass_utils

F32 = mybir.dt.float32
F32R = mybir.dt.float32r
BF16 = mybir.dt.bfloat16
AF = mybir.ActivationFunctionType

H, DK, DV, M, T, D = 16, 128, 128, 64, 2048, 2048
EPS, CLAMP, ROPE_BASE = 1e-5, 32.0, 10000.0
N_CORES = 8
NT = T // 128        # 16
NB = T // 512        # 4 big row-chunks
ND = D // 128        # 16
SCALE = DK ** -0.5

REPEAT = 1
DEBUG = False


def build(repeat=1, debug=False):
    nc = bacc.Bacc(None, target_bir_lowering=False, debug=False, num_devices=N_CORES)

    din = {}
    for nm, shp, dt in [
        ("hsb", [128, NB, ND, 512], BF16),
        ("wq0", [128, ND, 128], BF16), ("wq1", [128, ND, 128], BF16),
        ("wk0", [128, ND, 128], BF16), ("wk1", [128, ND, 128], BF16),
        ("wg0", [128, ND, 128], BF16), ("wg1", [128, ND, 128], BF16),
        ("wvu", [128, ND, 272], BF16),
        ("ws2e", [17, 128], BF16),
        ("ones2k", [1, T], BF16),
        ("onesrow_b", [1, 128], BF16), ("onescol_b", [128, 1], BF16),
        ("onesrow_r", [1, 128], F32R), ("onescol_r", [128, 1], F32R),
        ("cossin", [128, T], F32),
        ("triu", [128, 128], BF16), ("ident", [128, 128], F32),
        ("masks", [128, 4, 512], BF16),
        ("woT", [128, ND, D], BF16),
    ]:
        din[nm] = nc.dram_tensor(nm, shp, dt, kind="ExternalInput")
    out_d = nc.dram_tensor("out", [256, D], F32, kind="ExternalOutput")

    dbg = {}
    if debug:
        for nm, shp, dt in [("qT", [256, T], BF16), ("kT", [256, T], BF16),
                            ("v", [128, NT * 256], BF16),
                            ("sg", [256, T], BF16), ("es", [128, NT * 128], BF16),
                            ("enzT", [128, T], F32), ("esT", [128, T], BF16),
                            ("u17", [17, T], BF16), ("qveT", [128, T], BF16),
                            ("ogT", [256, T], BF16)]:
            dbg[nm] = nc.dram_tensor("dbg_" + nm, shp, dt, kind="ExternalOutput")

    with tile.TileContext(nc) as tc:
        with tc.tile_pool(name="const", bufs=1) as cpool, \
             tc.tile_pool(name="big", bufs=1) as big:
            c = {}
            for nm in ("ws2e", "onesrow_b", "onescol_b", "onesrow_r",
                       "onescol_r", "triu", "ident"):
                tl = cpool.tile(list(din[nm].shape), din[nm].dtype, tag=nm, name=nm)
                nc.sync.dma_start(tl[:], din[nm].ap())
                c[nm] = tl

            st = {
                "qT": [big.tile([128, T], BF16, tag=f"qT{h}", name=f"qT{h}") for h in range(2)],
                "kT": [big.tile([128, T], BF16, tag=f"kT{h}", name=f"kT{h}") for h in range(2)],
                "sg": [big.tile([128, T], BF16, tag=f"sg{h}", name=f"sg{h}") for h in range(2)],
                "v": big.tile([128, NT, 256], BF16, tag="v", name="v"),
                "u17": big.tile([17, T], BF16, tag="u17", name="u17"),
                "es_t": big.tile([128, NT, 128], BF16, tag="es_t", name="es_t"),
                "esT": big.tile([128, T], BF16, tag="esT", name="esT"),
                "enzT": big.tile([128, T], F32, tag="enzT", name="enzT"),
            }
            for _ in range(repeat):
                _pass(nc, tc, din, c, st, out_d, dbg, debug)

    nc.compile()
    return nc, dbg


def _pass(nc, tc, din, c, st, out_d, dbg, debug):
    qT, kT, sg = st["qT"], st["kT"], st["sg"]
    v, u17, es_t, esT, enzT = st["v"], st["u17"], st["es_t"], st["esT"], st["enzT"]

    # ================= PHASE 1: projections (4 x 512-col sweeps) =================
    with tc.tile_pool(name="p1w", bufs=1) as p1w, \
         tc.tile_pool(name="p1sb", bufs=2) as p1sb, \
         tc.tile_pool(name="p1hs", bufs=2) as p1hs:
        p1ps_cm = tc.tile_pool(name="p1ps", bufs=1, space="PSUM")
        p1ps = p1ps_cm.__enter__()
        # hs chunk 0 first (4 sub-tiles of 4 d-groups each), then weights:
        # the first matmul needs only hsq sub 0 + wq0, so compute starts
        # ~6us in instead of waiting for the full weight set
        def hs_load(tb):
            subs = []
            for s in range(4):
                t = p1hs.tile([128, 4, 512], BF16, tag=f"hsq{s}",
                              name=f"hsq{tb}_{s}")
                nc.sync.dma_start(t[:], din["hsb"].ap()[:, tb, 4 * s:4 * s + 4])
                subs.append(t)
            return subs

        hs_next = hs_load(0)
        wts = {}
        for wn in ("wq0", "wq1", "wk0", "wk1", "wg0", "wg1", "wvu"):
            cw = din[wn].shape[2]
            wt = p1w.tile([128, ND, cw], BF16, tag=wn, name=wn)
            nc.sync.dma_start(wt[:], din[wn].ap())
            wts[wn] = wt
        cossin = p1w.tile([128, T], F32, tag="cossin", name="cossin")
        nc.sync.dma_start(cossin[:], din["cossin"].ap())
        nc.sync.dma_start(u17[16:17, :], din["ones2k"].ap())

        carries = []
        QKG = (("wq0", qT[0], "q"), ("wq1", qT[1], "q"),
               ("wk0", kT[0], "k"), ("wk1", kT[1], "k"),
               ("wg0", sg[0], "g"), ("wg1", sg[1], "g"))
        for tb in range(NB):
            tsl = slice(tb * 512, (tb + 1) * 512)
            hsq_s = hs_next
            if tb + 1 < NB:
                hs_next = hs_load(tb + 1)
            acc = {wn: p1ps.tile([128, 512], F32, tag=f"acc_{wn}",
                                 name=f"acc_{wn}_{tb}")
                   for wn, *_ in QKG}
            accv = [p1ps.tile([128, 272], F32, tag=f"acc_v{i}", name=f"acc_v{i}_{tb}")
                    for i in range(2)]
            for d in range(ND):
                hs_d = hsq_s[d // 4][:, d % 4, :]
                for wn, dst, kind in QKG:
                    nc.tensor.matmul(acc[wn][:], wts[wn][:, d, :], hs_d,
                                     start=(d == 0), stop=(d == ND - 1))
                for i in range(2):
                    nc.tensor.matmul(accv[i][:],
                                     hs_d[:, i * 128:(i + 1) * 128],
                                     wts["wvu"][:, d, :],
                                     start=(d == 0), stop=(d == ND - 1))
            # drains: RoPE for q/k, silu for g
            for wn, dst, kind in QKG:
                ps = acc[wn]
                if kind == "g":
                    sgm = p1sb.tile([128, 512], F32, tag="sgm")
                    nc.scalar.activation(sgm[:], ps[:], AF.Sigmoid)
                    nc.vector.tensor_mul(dst[:, tsl], ps[:], sgm[:])
                else:
                    t1 = p1sb.tile([64, 512], F32, tag="ropet1")
                    t2 = p1sb.tile([64, 512], F32, tag="ropet2")
                    nc.vector.tensor_mul(t1[:], ps[0:64, :], cossin[0:64, tsl])
                    nc.vector.tensor_mul(t2[:], ps[64:128, :], cossin[64:128, tsl])
                    nc.vector.tensor_sub(dst[0:64, tsl], t1[:], t2[:])
                    nc.vector.tensor_mul(t1[:], ps[64:128, :], cossin[0:64, tsl])
                    nc.vector.tensor_mul(t2[:], ps[0:64, :], cossin[64:128, tsl])
                    nc.vector.tensor_add(dst[64:128, tsl], t1[:], t2[:])
            # v+u drains for subblocks 0,1; then second v+u pass for 2,3
            def drain_vu(i, accv_i):
                ts = 4 * tb + i
                nc.scalar.copy(v[:, ts, :], accv_i[:, 0:256])
                usb = p1sb.tile([128, 16], F32, tag="usb")
                nc.scalar.copy(usb[:], accv_i[:, 256:272])
                utp = p1ps.tile([16, 128], F32, tag="acc_wq0", name=f"utp{ts}")
                nc.tensor.transpose(utp[:], usb[:], c["ident"][:])
                nc.scalar.copy(u17[0:16, ts * 128:(ts + 1) * 128], utp[:])

            drain_vu(0, accv[0])
            drain_vu(1, accv[1])
            accv2 = [p1ps.tile([128, 272], F32, tag=f"acc_v{i}", name=f"acc_v2{i}_{tb}")
                     for i in range(2)]
            for d in range(ND):
                hs_d = hsq_s[d // 4][:, d % 4, :]
                for i in range(2):
                    nc.tensor.matmul(accv2[i][:],
                                     hs_d[:, (2 + i) * 128:(3 + i) * 128],
                                     wts["wvu"][:, d, :],
                                     start=(d == 0), stop=(d == ND - 1))
            drain_vu(2, accv2[0])
            drain_vu(3, accv2[1])

            # slot logits for this chunk: esT (col-oriented) + es_t/cumsum
            pse = p1ps.tile([128, 512], F32, tag="acc_wk0", name=f"psesT{tb}")
            nc.tensor.matmul(pse[:], c["ws2e"][:], u17[:, tsl], start=True, stop=True)
            nc.scalar.activation(esT[:, tsl], pse[:], AF.Exp)
            for ts in range(4 * tb, 4 * tb + 4):
                ssl = slice(ts * 128, (ts + 1) * 128)
                pss = p1ps.tile([128, 128], F32, tag="acc_wk1", name=f"ps_st{ts}")
                nc.tensor.matmul(pss[:], u17[:, ssl], c["ws2e"][:],
                                 start=True, stop=True)
                nc.scalar.activation(es_t[:, ts, :], pss[:], AF.Exp)
                cs2 = p1ps.tile([128, 128], F32, tag="acc_wg0", name=f"ps_cs{ts}")
                nc.tensor.matmul(cs2[:], c["triu"][:], es_t[:, ts, :],
                                 start=True, stop=(ts == 0))
                if ts > 0:
                    nc.tensor.matmul(cs2[:], c["onesrow_b"][:], carries[ts - 1][:],
                                     start=False, stop=True)
                enz_sb = p1sb.tile([128, 128], F32, tag="enz_sb")
                nc.vector.reciprocal_approx_fast(enz_sb[:], cs2[:])
                etp = p1ps.tile([128, 128], F32, tag="acc_wg1", name=f"etp{ts}")
                nc.tensor.transpose(etp[:], enz_sb[:], c["ident"][:])
                nc.scalar.copy(enzT[:, ssl], etp[:])
                if ts < NT - 1:
                    csum = p1ps.tile([1, 128], F32, tag="acc_wq1",
                                     name=f"csum{ts}")
                    nc.tensor.matmul(csum[:], c["onescol_b"][:], es_t[:, ts, :],
                                     start=True, stop=(ts == 0))
                    if ts > 0:
                        nc.tensor.matmul(csum[:], c["onesrow_b"][:, 0:1],
                                         carries[ts - 1][:], start=False, stop=True)
                    cr = p1sb.tile([1, 128], BF16, tag="carry", name=f"carry{ts}",
                                   bufs=4)
                    nc.vector.tensor_copy(cr[:], csum[:])
                    carries.append(cr)
        p1ps_cm.__exit__(None, None, None)

    if debug:
        for h in range(2):
            nc.sync.dma_start(dbg["qT"].ap()[h * 128:(h + 1) * 128, :], qT[h][:])
            nc.sync.dma_start(dbg["kT"].ap()[h * 128:(h + 1) * 128, :], kT[h][:])
            nc.sync.dma_start(dbg["sg"].ap()[h * 128:(h + 1) * 128, :], sg[h][:])
        nc.sync.dma_start(dbg["v"].ap(), v[:].rearrange("p a b -> p (a b)"))
        nc.sync.dma_start(dbg["es"].ap(), es_t[:].rearrange("p a b -> p (a b)"))
        nc.sync.dma_start(dbg["enzT"].ap(), enzT[:])
        nc.sync.dma_start(dbg["esT"].ap(), esT[:])
        nc.sync.dma_start(dbg["u17"].ap(), u17[:])

    # ================= PHASE 2: attention =================
    with tc.tile_pool(name="p2sb", bufs=3) as p2sb, \
         tc.tile_pool(name="qveP", bufs=2) as qvep, \
         tc.tile_pool(name="dramp", bufs=1, space="DRAM") as dpool:
        ag_inb = dpool.tile([N_CORES, 2, 128, 256], BF16, tag="ag_inb", name="ag_inb")
        ag_outb = dpool.tile([N_CORES, 2, 128, 256], BF16, tag="ag_outb", name="ag_outb")
        epsb = p2sb.tile([128, 1], F32, tag="epsb", bufs=1)
        nc.vector.memset(epsb[:], EPS)
        tl = p2sb.tile(list(din["masks"].shape), BF16, tag="masks", name="masks", bufs=1)
        nc.sync.dma_start(tl[:], din["masks"].ap())
        c["masks"] = tl
        # o_proj weights: prefetch the full 8MB during attention
        wo_sb = p2sb.tile([128, ND, D], BF16, tag="wo_sb", name="wo_sb", bufs=1)
        nc.sync.dma_start(wo_sb[:], din["woT"].ap())

        with tc.tile_pool(name="ps_at", bufs=3, space="PSUM") as ps_at, \
             tc.tile_pool(name="ps_ok", bufs=2, space="PSUM") as ps_ok, \
             tc.tile_pool(name="ps_ot", bufs=2, space="PSUM") as ps_ot, \
             tc.tile_pool(name="ps_aux", bufs=1, space="PSUM") as ps_aux:
            def stage1(I):
                rsl = slice(I * 512, (I + 1) * 512)
                njs = 4 * I + 4
                okT = ps_ok.tile([128, 512], F32, tag="okT", name=f"okT{I}",
                                 bufs=2)
                at_sb = {}

                def at_pair(j):
                    for h in range(2):
                        at = ps_at.tile([128, 512], F32, tag="atw2")
                        nc.tensor.matmul(at[:], kT[h][:, j * 128:(j + 1) * 128],
                                         qT[h][:, rsl], start=True, stop=True)
                        a_sb = p2sb.tile([128, 512], BF16, tag="at_sb")
                        m = j - 4 * I
                        if m >= 0:
                            nc.vector.tensor_mul(a_sb[:], at[:], c["masks"][:, m, :])
                        elif h == 0:
                            nc.scalar.copy(a_sb[:], at[:])
                        else:
                            nc.vector.tensor_copy(a_sb[:], at[:])
                        at_sb[(j, h)] = a_sb

                at_pair(0)
                for j in range(njs):
                    if j + 1 < njs:
                        at_pair(j + 1)
                    for h in range(2):
                        nc.tensor.matmul(okT[h * 64:(h + 1) * 64, :],
                                         es_t[:, j, h * 64:(h + 1) * 64],
                                         at_sb.pop((j, h))[:],
                                         start=(j == 0),
                                         stop=(j == njs - 1))
                return okT

            def softmax(I, okT):
                # unnormalized: qveT = exp(okT * enzT) * enzT  (RMS-norm
                # downstream cancels the per-(t,h) softmax denominator)
                rsl = slice(I * 512, (I + 1) * 512)
                qveT = qvep.tile([128, 512], BF16, tag="qveT")
                okm = p2sb.tile([128, 512], F32, tag="okm")
                nc.vector.tensor_mul(okm[:], okT[:], enzT[:, rsl])
                eok = p2sb.tile([128, 512], F32, tag="eok")
                nc.scalar.activation(eok[:], okm[:], AF.Exp)
                nc.vector.tensor_mul(qveT[:], eok[:], enzT[:, rsl])
                return qveT

            def stage2(I, qveT):
                rsl = slice(I * 512, (I + 1) * 512)
                njs = 4 * I + 4
                oT = [ps_ot.tile([128, 512], F32, tag="oT", name=f"oT{i}") for i in range(2)]
                w2_sb = {}

                def w2_pair(j):
                    for h in range(2):
                        w2 = ps_at.tile([128, 512], F32, tag="atw2")
                        nc.tensor.matmul(w2[:],
                                         esT[h * 64:(h + 1) * 64, j * 128:(j + 1) * 128],
                                         qveT[h * 64:(h + 1) * 64, :],
                                         start=True, stop=True)
                        wsb = p2sb.tile([128, 512], BF16, tag="at_sb")
                        m = j - 4 * I
                        if m >= 0:
                            nc.vector.tensor_mul(wsb[:], w2[:], c["masks"][:, m, :])
                        elif h == 0:
                            nc.scalar.copy(wsb[:], w2[:])
                        else:
                            nc.vector.tensor_copy(wsb[:], w2[:])
                        w2_sb[(j, h)] = wsb

                w2_pair(0)
                for j in range(njs):
                    if j + 1 < njs:
                        w2_pair(j + 1)
                    for h in range(2):
                        nc.tensor.matmul(oT[h][:], v[:, j, h * 128:(h + 1) * 128],
                                         w2_sb.pop((j, h))[:],
                                         start=(j == 0), stop=(j == njs - 1))
                # epilogue: o_g = o * rsqrt(mean o^2 + eps) * sg -> bf16 -> a2a_in
                for h in range(2):
                    sq = p2sb.tile([128, 512], F32R, tag="sq")
                    nc.scalar.activation(sq[:], oT[h][:], AF.Square)
                    ssq = ps_aux.tile([1, 512], F32, tag="aux")
                    nc.tensor.matmul(ssq[:], c["onescol_r"][:], sq[:], start=True, stop=True)
                    ssq_sb = p2sb.tile([1, 512], F32R, tag="ssq_sb")
                    with nc.allow_low_precision(reason="f32r bitcast for broadcast"):
                        nc.scalar.copy(ssq_sb[:], ssq[:])
                    rb = ps_aux.tile([128, 512], F32, tag="aux")
                    nc.tensor.matmul(rb[:], c["onesrow_r"][:], ssq_sb[:], start=True, stop=True)
                    rms = p2sb.tile([128, 512], F32, tag="rms")
                    nc.scalar.activation(rms[:], rb[:], AF.Sqrt, scale=1.0 / DV,
                                         bias=epsb[:])
                    rinv = p2sb.tile([128, 512], F32, tag="rinv")
                    nc.vector.reciprocal_approx_fast(rinv[:], rms[:])
                    t1 = p2sb.tile([128, 512], F32, tag="ept1")
                    nc.vector.tensor_mul(t1[:], oT[h][:], sg[h][:, rsl])
                    ogt = p2sb.tile([128, 512], BF16, tag="ogt")
                    nc.vector.tensor_mul(ogt[:], t1[:], rinv[:])
                    if debug:
                        nc.gpsimd.dma_start(dbg["ogT"].ap()[h * 128:(h + 1) * 128, rsl], ogt[:])
                    nc.sync.dma_start(ag_inb[2 * I, h, :, :], ogt[:, 0:256])
                    nc.sync.dma_start(ag_inb[2 * I + 1, h, :, :], ogt[:, 256:512])

            ok_prev = stage1(0)
            qv_prev = softmax(0, ok_prev)
            if debug:
                nc.sync.dma_start(dbg["qveT"].ap()[:, 0:512], qv_prev[:])
            for I in range(1, NB):
                okI = stage1(I)
                stage2(I - 1, qv_prev)
                qv_prev = softmax(I, okI)
                if debug:
                    nc.sync.dma_start(dbg["qveT"].ap()[:, I * 512:(I + 1) * 512], qv_prev[:])
            stage2(NB - 1, qv_prev)

        # ================= PHASE 3: per-head A2A + o_proj =================
        # A2A(h) starts once stage2(3)'s h epilogue DMAs land; o_proj h=0
        # matmuls overlap the h=1 transfer
        nc.gpsimd.collective_compute(
            "AllToAll", mybir.AluOpType.bypass,
            replica_groups=[list(range(N_CORES))],
            ins=[ag_inb[:].opt()], outs=[ag_outb[:].opt()])
        og = {}
        for h in range(2):
            for s in range(N_CORES):
                ot = p2sb.tile([128, 256], BF16, tag=f"og{h}{s}",
                               name=f"og{h}{s}", bufs=1)
                nc.sync.dma_start(ot[:], ag_outb[s, h, :, :])
                og[(h, s)] = ot
        p3ps_cm = tc.tile_pool(name="p3ps", bufs=1, space="PSUM")
        p3ps = p3ps_cm.__enter__()
        for th in range(2):
            for ns in range(4):
                pso = p3ps.tile([128, 512], F32, tag=f"pso{th}{ns}",
                                name=f"pso{th}{ns}", bufs=1)
                n = 0
                for h in range(2):
                    for s in range(N_CORES):
                        kc = 2 * s + h
                        nc.tensor.matmul(pso[:],
                                         og[(h, s)][:, th * 128:(th + 1) * 128],
                                         wo_sb[:, kc, ns * 512:(ns + 1) * 512],
                                         start=(n == 0), stop=(n == 15))
                        n += 1
                osb = p2sb.tile([128, 512], F32, tag="osb")
                nc.scalar.copy(osb[:], pso[:])
                nc.sync.dma_start(
                    out_d.ap()[th * 128:(th + 1) * 128, ns * 512:(ns + 1) * 512],
                    osb[:])
        p3ps_cm.__exit__(None, None, None)


# ======================= host side =======================

def _host_inputs(inputs):
    import ml_dtypes
    BF = ml_dtypes.bfloat16
    hs = np.ascontiguousarray(np.asarray(inputs["hidden_states"], np.float32)[0])
    Wq = np.asarray(inputs["Wq"], np.float32)
    Wk = np.asarray(inputs["Wk"], np.float32)
    Wv = np.asarray(inputs["Wv"], np.float32)
    Wg = np.asarray(inputs["Wg"], np.float32)
    Wo = np.asarray(inputs["Wo"], np.float32)
    Ws1 = np.asarray(inputs["Ws1"], np.float32)
    Ws2 = np.asarray(inputs["Ws2"], np.float32)
    bs2 = np.asarray(inputs["bs2"], np.float32)
    gnw = np.asarray(inputs["g_norm_weight"], np.float32)

    hsT = hs.T  # [D, T]
    # hsb: [p, chunk, k, t] with d = k*128 + p
    hsb = np.ascontiguousarray(
        hsT.reshape(ND, 128, NB, 512).transpose(1, 2, 0, 3)).astype(BF)
    pos = np.arange(T, dtype=np.float64)
    inv = 1.0 / (ROPE_BASE ** (np.arange(0, DK, 2, dtype=np.float64) / DK))
    ang = pos[:, None] * inv[None, :]
    cos = np.cos(ang).T.astype(np.float32)       # [64, T]
    sin = np.sin(ang).T.astype(np.float32)
    cossin = np.concatenate([cos, sin], axis=0).astype(np.float32)
    triu = np.triu(np.ones((128, 128), np.float32)).astype(BF)
    masks = np.zeros((128, 4, 512), np.float32)
    p = np.arange(128)[:, None]
    r = np.arange(512)[None, :]
    for m in range(4):
        masks[:, m, :] = (128 * m + p <= r).astype(np.float32)
    ident = np.eye(128, dtype=np.float32)
    onesrow = np.ones((1, 128), np.float32)
    ones2k = np.ones((1, T), np.float32).astype(BF)
    onescol = np.ones((128, 1), np.float32)
    # woT: [p, kc, n] with hd = kc*128 + p; gnw folded in
    woT = (Wo.T * np.tile(gnw, H)[:, None]).astype(BF)
    woT = np.ascontiguousarray(woT.reshape(ND, 128, D).transpose(1, 0, 2))

    def wlay(w):  # [2048, 128] -> [p, k, c] bf16
        return np.ascontiguousarray(
            w.reshape(ND, 128, -1).transpose(1, 0, 2)).astype(BF)

    in_maps = []
    for core in range(N_CORES):
        sl = slice(core * 256, (core + 1) * 256)
        ssl = slice(core * 128, (core + 1) * 128)
        ws2e = np.concatenate([Ws2[ssl].T, bs2[None, ssl]], axis=0).astype(BF)
        wvu = np.concatenate([Wv[sl].T, Ws1.T], axis=1)  # [2048, 272]
        m = {
            "hsb": hsb,
            "wq0": wlay(Wq[sl].T[:, 0:128] * SCALE),
            "wq1": wlay(Wq[sl].T[:, 128:256] * SCALE),
            "wk0": wlay(Wk[sl].T[:, 0:128]),
            "wk1": wlay(Wk[sl].T[:, 128:256]),
            "wg0": wlay(Wg[sl].T[:, 0:128]),
            "wg1": wlay(Wg[sl].T[:, 128:256]),
            "wvu": wlay(wvu),
            "ws2e": ws2e,
            "onesrow_b": onesrow.astype(BF), "onescol_b": onescol.astype(BF),
            "onesrow_r": onesrow, "onescol_r": onescol,
            "ones2k": ones2k,
            "cossin": cossin,
            "triu": triu, "masks": masks.astype(BF), "ident": ident,
            "woT": woT,
        }
        in_maps.append(m)
    return in_maps


_CACHE = {}


def kernel(**inputs):
    key = ("k", REPEAT, DEBUG)
    if key not in _CACHE:
        _CACHE[key] = build(repeat=REPEAT, debug=DEBUG)
    nc, dbg = _CACHE[key]
    in_maps = _host_inputs(inputs)
    res = bass_utils.run_bass_kernel_spmd(nc, in_maps, core_ids=list(range(N_CORES)))
    out = np.concatenate([res.results[c]["out"] for c in range(N_CORES)], axis=0)
    kernel.last_results = res
    return out.reshape(1, T, D).astype(np.float32)


# revision 18
# speedup vs baseline: 1.5276x; 1.0409x over previous
"""ABC attention (gated slot attention) on 8 TRN2 NeuronCores.

Sharding: 2 heads per core (16 heads / 8 cores). Per core:
  - projections q,k (RoPE, q pre-scaled), v, silu(gate), slot logits,
    all matmuls bf16 (2x stream rate vs f32r), moving dim 512
  - unnormalized softmax: RMS-norm downstream is scale-invariant, so
    softmax keeps only exp(ok*enz)*enz; enz applied in [m,t] layout
    (enzT) -> no per-row transposes/reductions/reciprocals
  - quadratic chunked ABC attention, causal masking via mask tiles
  - fused RMS-norm x gate epilogue (Rsqrt broadcast via PE)
  - AllToAll reshards o_g head-split -> T-split (1MB/core vs 8.4MB
    AllGather); per-core o_proj over its 256-row T slice.
"""
import sys
if '/opt/trn_rl_repo' not in sys.path:
    sys.path.insert(0, '/opt/trn_rl_repo')
import numpy as np

import concourse.bacc as bacc
import concourse.mybir as mybir
import concourse.tile as tile
from concourse import bass_utils

F32 = mybir.dt.float32
F32R = mybir.dt.float32r
BF16 = mybir.dt.bfloat16
AF = mybir.ActivationFunctionType

H, DK, DV, M, T, D = 16, 128, 128, 64, 2048, 2048
EPS, CLAMP, ROPE_BASE = 1e-5, 32.0, 10000.0
N_CORES = 8
NT = T // 128        # 16
NB = T // 512        # 4 big row-chunks
ND = D // 128        # 16
SCALE = DK ** -0.5

REPEAT = 1
DEBUG = False


def build(repeat=1, debug=False):
    nc = bacc.Bacc(None, target_bir_lowering=False, debug=False, num_devices=N_CORES)

    din = {}
    for nm, shp, dt in [
        ("hsb", [128, NB, ND, 512], BF16),
        ("wq0", [128, ND, 128], BF16), ("wq1", [128, ND, 128], BF16),
        ("wk0", [128, ND, 128], BF16), ("wk1", [128, ND, 128], BF16),
        ("wg0", [128, ND, 128], BF16), ("wg1", [128, ND, 128], BF16),
        ("wvu", [128, ND, 272], BF16),
        ("ws2e", [17, 128], BF16),
        ("ones2k", [1, T], BF16),
        ("onesrow_b", [1, 128], BF16), ("onescol_b", [128, 1], BF16),
        ("onesrow_r", [1, 128], F32R), ("onescol_r", [128, 1], F32R),
        ("cossin", [128, T], F32),
        ("triu", [128, 128], BF16), ("ident", [128, 128], F32),
        ("masks", [128, 4, 512], BF16),
        ("woT", [128, ND, D], BF16),
    ]:
        din[nm] = nc.dram_tensor(nm, shp, dt, kind="ExternalInput")
    out_d = nc.dram_tensor("out", [256, D], F32, kind="ExternalOutput")

    dbg = {}
    if debug:
        for nm, shp, dt in [("qT", [256, T], BF16), ("kT", [256, T], BF16),
                            ("v", [128, NT * 256], BF16),
                            ("sg", [256, T], BF16), ("es", [128, NT * 128], BF16),
                            ("enzT", [128, T], F32), ("esT", [128, T], BF16),
                            ("u17", [17, T], BF16), ("qveT", [128, T], BF16),
                            ("ogT", [256, T], BF16)]:
            dbg[nm] = nc.dram_tensor("dbg_" + nm, shp, dt, kind="ExternalOutput")

    with tile.TileContext(nc) as tc:
        with tc.tile_pool(name="const", bufs=1) as cpool, \
             tc.tile_pool(name="big", bufs=1) as big:
            c = {}
            for nm in ("ws2e", "onesrow_b", "onescol_b", "onesrow_r",
                       "onescol_r", "triu", "ident"):
                tl = cpool.tile(list(din[nm].shape), din[nm].dtype, tag=nm, name=nm)
                nc.sync.dma_start(tl[:], din[nm].ap())
                c[nm] = tl

            st = {
                "qT": [big.tile([128, T], BF16, tag=f"qT{h}", name=f"qT{h}") for h in range(2)],
                "kT": [big.tile([128, T], BF16, tag=f"kT{h}", name=f"kT{h}") for h in range(2)],
                "sg": [big.tile([128, T], BF16, tag=f"sg{h}", name=f"sg{h}") for h in range(2)],
                "v": big.tile([128, NT, 256], BF16, tag="v", name="v"),
                "u17": big.tile([17, T], BF16, tag="u17", name="u17"),
                "es_t": big.tile([128, NT, 128], BF16, tag="es_t", name="es_t"),
                "esT": big.tile([128, T], BF16, tag="esT", name="esT"),
                "enzT": big.tile([128, T], F32, tag="enzT", name="enzT"),
            }
            for _ in range(repeat):
                _pass(nc, tc, din, c, st, out_d, dbg, debug)

    nc.compile()
    return nc, dbg


def _pass(nc, tc, din, c, st, out_d, dbg, debug):
    qT, kT, sg = st["qT"], st["kT"], st["sg"]
    v, u17, es_t, esT, enzT = st["v"], st["u17"], st["es_t"], st["esT"], st["enzT"]

    # ================= PHASE 1: projections (4 x 512-col sweeps) =================
    with tc.tile_pool(name="p1w", bufs=1) as p1w, \
         tc.tile_pool(name="p1sb", bufs=2) as p1sb, \
         tc.tile_pool(name="p1hs", bufs=2) as p1hs:
        p1ps_cm = tc.tile_pool(name="p1ps", bufs=1, space="PSUM")
        p1ps = p1ps_cm.__enter__()
        # hs chunk 0 first (4 sub-tiles of 4 d-groups each), then weights:
        # the first matmul needs only hsq sub 0 + wq0, so compute starts
        # ~6us in instead of waiting for the full weight set
        def hs_load(tb):
            subs = []
            for s in range(4):
                t = p1hs.tile([128, 4, 512], BF16, tag=f"hsq{s}",
                              name=f"hsq{tb}_{s}")
                nc.sync.dma_start(t[:], din["hsb"].ap()[:, tb, 4 * s:4 * s + 4])
                subs.append(t)
            return subs

        hs_next = hs_load(0)
        wts = {}
        for wn in ("wq0", "wq1", "wk0", "wk1", "wg0", "wg1", "wvu"):
            cw = din[wn].shape[2]
            wt = p1w.tile([128, ND, cw], BF16, tag=wn, name=wn)
            nc.sync.dma_start(wt[:], din[wn].ap())
            wts[wn] = wt
        cossin = p1w.tile([128, T], F32, tag="cossin", name="cossin")
        nc.sync.dma_start(cossin[:], din["cossin"].ap())
        nc.sync.dma_start(u17[16:17, :], din["ones2k"].ap())

        carries = []
        QKG = (("wq0", qT[0], "q"), ("wq1", qT[1], "q"),
               ("wk0", kT[0], "k"), ("wk1", kT[1], "k"),
               ("wg0", sg[0], "g"), ("wg1", sg[1], "g"))
        for tb in range(NB):
            tsl = slice(tb * 512, (tb + 1) * 512)
            hsq_s = hs_next
            if tb + 1 < NB:
                hs_next = hs_load(tb + 1)
            acc = {wn: p1ps.tile([128, 512], F32, tag=f"acc_{wn}",
                                 name=f"acc_{wn}_{tb}")
                   for wn, *_ in QKG}
            accv = [p1ps.tile([128, 272], F32, tag=f"acc_v{i}", name=f"acc_v{i}_{tb}")
                    for i in range(2)]
            for d in range(ND):
                hs_d = hsq_s[d // 4][:, d % 4, :]
                for wn, dst, kind in QKG:
                    nc.tensor.matmul(acc[wn][:], wts[wn][:, d, :], hs_d,
                                     start=(d == 0), stop=(d == ND - 1))
                for i in range(2):
                    nc.tensor.matmul(accv[i][:],
                                     hs_d[:, i * 128:(i + 1) * 128],
                                     wts["wvu"][:, d, :],
                                     start=(d == 0), stop=(d == ND - 1))
            # drains: RoPE for q/k, silu for g
            for wn, dst, kind in QKG:
                ps = acc[wn]
                if kind == "g":
                    sgm = p1sb.tile([128, 512], F32, tag="sgm")
                    nc.scalar.activation(sgm[:], ps[:], AF.Sigmoid)
                    nc.vector.tensor_mul(dst[:, tsl], ps[:], sgm[:])
                else:
                    t1 = p1sb.tile([64, 512], F32, tag="ropet1")
                    t2 = p1sb.tile([64, 512], F32, tag="ropet2")
                    nc.vector.tensor_mul(t1[:], ps[0:64, :], cossin[0:64, tsl])
                    nc.vector.tensor_mul(t2[:], ps[64:128, :], cossin[64:128, tsl])
                    nc.vector.tensor_sub(dst[0:64, tsl], t1[:], t2[:])
                    nc.vector.tensor_mul(t1[:], ps[64:128, :], cossin[0:64, tsl])
                    nc.vector.tensor_mul(t2[:], ps[0:64, :], cossin[64:128, tsl])
                    nc.vector.tensor_add(dst[64:128, tsl], t1[:], t2[:])
            # v+u drains for subblocks 0,1; then second v+u pass for 2,3
            def drain_vu(i, accv_i):
                ts = 4 * tb + i
                nc.scalar.copy(v[:, ts, :], accv_i[:, 0:256])
                usb = p1sb.tile([128, 16], F32, tag="usb")
                nc.scalar.copy(usb[:], accv_i[:, 256:272])
                utp = p1ps.tile([16, 128], F32, tag="acc_wq0", name=f"utp{ts}")
                nc.tensor.transpose(utp[:], usb[:], c["ident"][:])
                nc.scalar.copy(u17[0:16, ts * 128:(ts + 1) * 128], utp[:])

            drain_vu(0, accv[0])
            drain_vu(1, accv[1])
            accv2 = [p1ps.tile([128, 272], F32, tag=f"acc_v{i}", name=f"acc_v2{i}_{tb}")
                     for i in range(2)]
            for d in range(ND):
                hs_d = hsq_s[d // 4][:, d % 4, :]
                for i in range(2):
                    nc.tensor.matmul(accv2[i][:],
                                     hs_d[:, (2 + i) * 128:(3 + i) * 128],
                                     wts["wvu"][:, d, :],
                                     start=(d == 0), stop=(d == ND - 1))
            drain_vu(2, accv2[0])
            drain_vu(3, accv2[1])

            # slot logits for this chunk: esT (col-oriented) + es_t/cumsum
            pse = p1ps.tile([128, 512], F32, tag="acc_wk0", name=f"psesT{tb}")
            nc.tensor.matmul(pse[:], c["ws2e"][:], u17[:, tsl], start=True, stop=True)
            nc.scalar.activation(esT[:, tsl], pse[:], AF.Exp)
            for ts in range(4 * tb, 4 * tb + 4):
                ssl = slice(ts * 128, (ts + 1) * 128)
                pss = p1ps.tile([128, 128], F32, tag="acc_wk1", name=f"ps_st{ts}")
                nc.tensor.matmul(pss[:], u17[:, ssl], c["ws2e"][:],
                                 start=True, stop=True)
                nc.scalar.activation(es_t[:, ts, :], pss[:], AF.Exp)
                cs2 = p1ps.tile([128, 128], F32, tag="acc_wg0", name=f"ps_cs{ts}")
                nc.tensor.matmul(cs2[:], c["triu"][:], es_t[:, ts, :],
                                 start=True, stop=(ts == 0))
                if ts > 0:
                    nc.tensor.matmul(cs2[:], c["onesrow_b"][:], carries[ts - 1][:],
                                     start=False, stop=True)
                enz_sb = p1sb.tile([128, 128], F32, tag="enz_sb")
                nc.vector.reciprocal_approx_fast(enz_sb[:], cs2[:])
                etp = p1ps.tile([128, 128], F32, tag="acc_wg1", name=f"etp{ts}")
                nc.tensor.transpose(etp[:], enz_sb[:], c["ident"][:])
                nc.scalar.copy(enzT[:, ssl], etp[:])
                if ts < NT - 1:
                    csum = p1ps.tile([1, 128], F32, tag="acc_wq1",
                                     name=f"csum{ts}")
                    nc.tensor.matmul(csum[:], c["onescol_b"][:], es_t[:, ts, :],
                                     start=True, stop=(ts == 0))
                    if ts > 0:
                        nc.tensor.matmul(csum[:], c["onesrow_b"][:, 0:1],
                                         carries[ts - 1][:], start=False, stop=True)
                    cr = p1sb.tile([1, 128], BF16, tag="carry", name=f"carry{ts}",
                                   bufs=4)
                    nc.vector.tensor_copy(cr[:], csum[:])
                    carries.append(cr)
        p1ps_cm.__exit__(None, None, None)

    if debug:
        for h in range(2):
            nc.sync.dma_start(dbg["qT"].ap()[h * 128:(h + 1) * 128, :], qT[h][:])
            nc.sync.dma_start(dbg["kT"].ap()[h * 128:(h + 1) * 128, :], kT[h][:])
            nc.sync.dma_start(dbg["sg"].ap()[h * 128:(h + 1) * 128, :], sg[h][:])
        nc.sync.dma_start(dbg["v"].ap(), v[:].rearrange("p a b -> p (a b)"))
        nc.sync.dma_start(dbg["es"].ap(), es_t[:].rearrange("p a b -> p (a b)"))
        nc.sync.dma_start(dbg["enzT"].ap(), enzT[:])
        nc.sync.dma_start(dbg["esT"].ap(), esT[:])
        nc.sync.dma_start(dbg["u17"].ap(), u17[:])

    # ================= PHASE 2: attention =================
    with tc.tile_pool(name="p2sb", bufs=3) as p2sb, \
         tc.tile_pool(name="qveP", bufs=2) as qvep, \
         tc.tile_pool(name="dramp", bufs=1, space="DRAM") as dpool:
        ag_in, ag_out = [], []
        for h in range(2):
            ag_in.append(dpool.tile([N_CORES, 128, 256], BF16,
                                    tag=f"ag_in{h}", name=f"ag_in{h}"))
            ag_out.append(dpool.tile([N_CORES, 128, 256], BF16,
                                     tag=f"ag_out{h}", name=f"ag_out{h}"))
        warm_in = dpool.tile([N_CORES, 16], BF16, tag="warm_in", name="warm_in")
        warm_out = dpool.tile([N_CORES, 16], BF16, tag="warm_out", name="warm_out")
        wsrc = p2sb.tile([1, 16 * N_CORES], BF16, tag="wsrc", bufs=1)
        nc.vector.memset(wsrc[:], 0.0)
        nc.sync.dma_start(warm_in[:].rearrange("s t -> (s t)"), wsrc[0])
        epsb = p2sb.tile([128, 1], F32, tag="epsb", bufs=1)
        nc.vector.memset(epsb[:], EPS)
        tl = p2sb.tile(list(din["masks"].shape), BF16, tag="masks", name="masks", bufs=1)
        nc.sync.dma_start(tl[:], din["masks"].ap())
        c["masks"] = tl
        # o_proj weights: prefetch the full 8MB during attention
        wo_sb = p2sb.tile([128, ND, D], BF16, tag="wo_sb", name="wo_sb", bufs=1)
        nc.sync.dma_start(wo_sb[:], din["woT"].ap())

        with tc.tile_pool(name="ps_at", bufs=3, space="PSUM") as ps_at, \
             tc.tile_pool(name="ps_ok", bufs=2, space="PSUM") as ps_ok, \
             tc.tile_pool(name="ps_ot", bufs=2, space="PSUM") as ps_ot, \
             tc.tile_pool(name="ps_aux", bufs=1, space="PSUM") as ps_aux:
            def stage1(I):
                rsl = slice(I * 512, (I + 1) * 512)
                njs = 4 * I + 4
                okT = ps_ok.tile([128, 512], F32, tag="okT", name=f"okT{I}",
                                 bufs=2)
                at_sb = {}

                def at_pair(j):
                    for h in range(2):
                        at = ps_at.tile([128, 512], F32, tag="atw2")
                        nc.tensor.matmul(at[:], kT[h][:, j * 128:(j + 1) * 128],
                                         qT[h][:, rsl], start=True, stop=True)
                        a_sb = p2sb.tile([128, 512], BF16, tag="at_sb")
                        m = j - 4 * I
                        if m >= 0:
                            nc.vector.tensor_mul(a_sb[:], at[:], c["masks"][:, m, :])
                        elif h == 0:
                            nc.scalar.copy(a_sb[:], at[:])
                        else:
                            nc.vector.tensor_copy(a_sb[:], at[:])
                        at_sb[(j, h)] = a_sb

                at_pair(0)
                for j in range(njs):
                    if j + 1 < njs:
                        at_pair(j + 1)
                    for h in range(2):
                        nc.tensor.matmul(okT[h * 64:(h + 1) * 64, :],
                                         es_t[:, j, h * 64:(h + 1) * 64],
                                         at_sb.pop((j, h))[:],
                                         start=(j == 0),
                                         stop=(j == njs - 1))
                return okT

            def softmax(I, okT):
                # unnormalized: qveT = exp(okT * enzT) * enzT  (RMS-norm
                # downstream cancels the per-(t,h) softmax denominator)
                rsl = slice(I * 512, (I + 1) * 512)
                qveT = qvep.tile([128, 512], BF16, tag="qveT")
                okm = p2sb.tile([128, 512], F32, tag="okm")
                nc.vector.tensor_mul(okm[:], okT[:], enzT[:, rsl])
                eok = p2sb.tile([128, 512], F32, tag="eok")
                nc.scalar.activation(eok[:], okm[:], AF.Exp)
                nc.vector.tensor_mul(qveT[:], eok[:], enzT[:, rsl])
                return qveT

            def stage2(I, qveT):
                rsl = slice(I * 512, (I + 1) * 512)
                njs = 4 * I + 4
                oT = [ps_ot.tile([128, 512], F32, tag="oT", name=f"oT{i}") for i in range(2)]
                w2_sb = {}

                def w2_pair(j):
                    for h in range(2):
                        w2 = ps_at.tile([128, 512], F32, tag="atw2")
                        nc.tensor.matmul(w2[:],
                                         esT[h * 64:(h + 1) * 64, j * 128:(j + 1) * 128],
                                         qveT[h * 64:(h + 1) * 64, :],
                                         start=True, stop=True)
                        wsb = p2sb.tile([128, 512], BF16, tag="at_sb")
                        m = j - 4 * I
                        if m >= 0:
                            nc.vector.tensor_mul(wsb[:], w2[:], c["masks"][:, m, :])
                        elif h == 0:
                            nc.scalar.copy(wsb[:], w2[:])
                        else:
                            nc.vector.tensor_copy(wsb[:], w2[:])
                        w2_sb[(j, h)] = wsb

                w2_pair(0)
                for j in range(njs):
                    if j + 1 < njs:
                        w2_pair(j + 1)
                    for h in range(2):
                        nc.tensor.matmul(oT[h][:], v[:, j, h * 128:(h + 1) * 128],
                                         w2_sb.pop((j, h))[:],
                                         start=(j == 0), stop=(j == njs - 1))
                # epilogue: o_g = o * rsqrt(mean o^2 + eps) * sg -> bf16 -> a2a_in
                for h in range(2):
                    sq = p2sb.tile([128, 512], F32R, tag="sq")
                    nc.scalar.activation(sq[:], oT[h][:], AF.Square)
                    ssq = ps_aux.tile([1, 512], F32, tag="aux")
                    nc.tensor.matmul(ssq[:], c["onescol_r"][:], sq[:], start=True, stop=True)
                    ssq_sb = p2sb.tile([1, 512], F32R, tag="ssq_sb")
                    with nc.allow_low_precision(reason="f32r bitcast for broadcast"):
                        nc.scalar.copy(ssq_sb[:], ssq[:])
                    rb = ps_aux.tile([128, 512], F32, tag="aux")
                    nc.tensor.matmul(rb[:], c["onesrow_r"][:], ssq_sb[:], start=True, stop=True)
                    rms = p2sb.tile([128, 512], F32, tag="rms")
                    nc.scalar.activation(rms[:], rb[:], AF.Sqrt, scale=1.0 / DV,
                                         bias=epsb[:])
                    rinv = p2sb.tile([128, 512], F32, tag="rinv")
                    nc.vector.reciprocal_approx_fast(rinv[:], rms[:])
                    t1 = p2sb.tile([128, 512], F32, tag="ept1")
                    nc.vector.tensor_mul(t1[:], oT[h][:], sg[h][:, rsl])
                    ogt = p2sb.tile([128, 512], BF16, tag="ogt")
                    nc.vector.tensor_mul(ogt[:], t1[:], rinv[:])
                    if debug:
                        nc.gpsimd.dma_start(dbg["ogT"].ap()[h * 128:(h + 1) * 128, rsl], ogt[:])
                    nc.sync.dma_start(ag_in[h][2 * I, :, :], ogt[:, 0:256])
                    nc.sync.dma_start(ag_in[h][2 * I + 1, :, :], ogt[:, 256:512])

            ok_prev = stage1(0)
            qv_prev = softmax(0, ok_prev)
            if debug:
                nc.sync.dma_start(dbg["qveT"].ap()[:, 0:512], qv_prev[:])
            for I in range(1, NB):
                okI = stage1(I)
                if I == 2:
                    # tiny warm-up A2A: keeps collective channels hot so the
                    # real transfers at the end run at steady-state latency
                    nc.gpsimd.collective_compute(
                        "AllToAll", mybir.AluOpType.bypass,
                        replica_groups=[list(range(N_CORES))],
                        ins=[warm_in[:].opt()], outs=[warm_out[:].opt()])
                stage2(I - 1, qv_prev)
                qv_prev = softmax(I, okI)
                if debug:
                    nc.sync.dma_start(dbg["qveT"].ap()[:, I * 512:(I + 1) * 512], qv_prev[:])
            stage2(NB - 1, qv_prev)

        # ================= PHASE 3: per-head A2A + o_proj =================
        # A2A(h) starts once stage2(3)'s h epilogue DMAs land; o_proj h=0
        # matmuls overlap the h=1 transfer
        for h in range(2):
            nc.gpsimd.collective_compute(
                "AllToAll", mybir.AluOpType.bypass,
                replica_groups=[list(range(N_CORES))],
                ins=[ag_in[h][:].opt()], outs=[ag_out[h][:].opt()])
        og = {}
        for h in range(2):
            for s in range(N_CORES):
                ot = p2sb.tile([128, 256], BF16, tag=f"og{h}{s}",
                               name=f"og{h}{s}", bufs=1)
                nc.sync.dma_start(ot[:], ag_out[h][s, :, :])
                og[(h, s)] = ot
        p3ps_cm = tc.tile_pool(name="p3ps", bufs=1, space="PSUM")
        p3ps = p3ps_cm.__enter__()
        for th in range(2):
            for ns in range(4):
                pso = p3ps.tile([128, 512], F32, tag=f"pso{th}{ns}",
                                name=f"pso{th}{ns}", bufs=1)
                n = 0
                for h in range(2):
                    for s in range(N_CORES):
                        kc = 2 * s + h
                        nc.tensor.matmul(pso[:],
                                         og[(h, s)][:, th * 128:(th + 1) * 128],
                                         wo_sb[:, kc, ns * 512:(ns + 1) * 512],
                                         start=(n == 0), stop=(n == 15))
                        n += 1
                osb = p2sb.tile([128, 512], F32, tag="osb")
                nc.scalar.copy(osb[:], pso[:])
                nc.sync.dma_start(
                    out_d.ap()[th * 128:(th + 1) * 128, ns * 512:(ns + 1) * 512],
                    osb[:])
        p3ps_cm.__exit__(None, None, None)


# ======================= host side =======================

def _host_inputs(inputs):
    import ml_dtypes
    BF = ml_dtypes.bfloat16
    hs = np.ascontiguousarray(np.asarray(inputs["hidden_states"], np.float32)[0])
    Wq = np.asarray(inputs["Wq"], np.float32)
    Wk = np.asarray(inputs["Wk"], np.float32)
    Wv = np.asarray(inputs["Wv"], np.float32)
    Wg = np.asarray(inputs["Wg"], np.float32)
    Wo = np.asarray(inputs["Wo"], np.float32)
    Ws1 = np.asarray(inputs["Ws1"], np.float32)
    Ws2 = np.asarray(inputs["Ws2"], np.float32)
    bs2 = np.asarray(inputs["bs2"], np.float32)
    gnw = np.asarray(inputs["g_norm_weight"], np.float32)

    hsT = hs.T  # [D, T]
    # hsb: [p, chunk, k, t] with d = k*128 + p
    hsb = np.ascontiguousarray(
        hsT.reshape(ND, 128, NB, 512).transpose(1, 2, 0, 3)).astype(BF)
    pos = np.arange(T, dtype=np.float64)
    inv = 1.0 / (ROPE_BASE ** (np.arange(0, DK, 2, dtype=np.float64) / DK))
    ang = pos[:, None] * inv[None, :]
    cos = np.cos(ang).T.astype(np.float32)       # [64, T]
    sin = np.sin(ang).T.astype(np.float32)
    cossin = np.concatenate([cos, sin], axis=0).astype(np.float32)
    triu = np.triu(np.ones((128, 128), np.float32)).astype(BF)
    masks = np.zeros((128, 4, 512), np.float32)
    p = np.arange(128)[:, None]
    r = np.arange(512)[None, :]
    for m in range(4):
        masks[:, m, :] = (128 * m + p <= r).astype(np.float32)
    ident = np.eye(128, dtype=np.float32)
    onesrow = np.ones((1, 128), np.float32)
    ones2k = np.ones((1, T), np.float32).astype(BF)
    onescol = np.ones((128, 1), np.float32)
    # woT: [p, kc, n] with hd = kc*128 + p; gnw folded in
    woT = (Wo.T * np.tile(gnw, H)[:, None]).astype(BF)
    woT = np.ascontiguousarray(woT.reshape(ND, 128, D).transpose(1, 0, 2))

    def wlay(w):  # [2048, 128] -> [p, k, c] bf16
        return np.ascontiguousarray(
            w.reshape(ND, 128, -1).transpose(1, 0, 2)).astype(BF)

    in_maps = []
    for core in range(N_CORES):
        sl = slice(core * 256, (core + 1) * 256)
        ssl = slice(core * 128, (core + 1) * 128)
        ws2e = np.concatenate([Ws2[ssl].T, bs2[None, ssl]], axis=0).astype(BF)
        wvu = np.concatenate([Wv[sl].T, Ws1.T], axis=1)  # [2048, 272]
        m = {
            "hsb": hsb,
            "wq0": wlay(Wq[sl].T[:, 0:128] * SCALE),
            "wq1": wlay(Wq[sl].T[:, 128:256] * SCALE),
            "wk0": wlay(Wk[sl].T[:, 0:128]),
            "wk1": wlay(Wk[sl].T[:, 128:256]),
            "wg0": wlay(Wg[sl].T[:, 0:128]),
            "wg1": wlay(Wg[sl].T[:, 128:256]),
            "wvu": wlay(wvu),
            "ws2e": ws2e,
            "onesrow_b": onesrow.astype(BF), "onescol_b": onescol.astype(BF),
            "onesrow_r": onesrow, "onescol_r": onescol,
            "ones2k": ones2k,
            "cossin": cossin,
            "triu": triu, "masks": masks.astype(BF), "ident": ident,
            "woT": woT,
        }
        in_maps.append(m)
    return in_maps


_CACHE = {}


def kernel(**inputs):
    key = ("k", REPEAT, DEBUG)
    if key not in _CACHE:
        _CACHE[key] = build(repeat=REPEAT, debug=DEBUG)
    nc, dbg = _CACHE[key]
    in_maps = _host_inputs(inputs)
    res = bass_utils.run_bass_kernel_spmd(nc, in_maps, core_ids=list(range(N_CORES)))
    out = np.concatenate([res.results[c]["out"] for c in range(N_CORES)], axis=0)
    kernel.last_results = res
    return out.reshape(1, T, D).astype(np.float32)


# revision 22
# speedup vs baseline: 1.5643x; 1.0241x over previous
"""ABC attention (gated slot attention) on 8 TRN2 NeuronCores.

Sharding: 2 heads per core (16 heads / 8 cores). Per core:
  - projections q,k (RoPE, q pre-scaled), v, silu(gate), slot logits,
    all matmuls bf16 (2x stream rate vs f32r), moving dim 512
  - unnormalized softmax: RMS-norm downstream is scale-invariant, so
    softmax keeps only exp(ok*enz)*enz; enz applied in [m,t] layout
    (enzT) -> no per-row transposes/reductions/reciprocals
  - quadratic chunked ABC attention, causal masking via mask tiles
  - fused RMS-norm x gate epilogue (Rsqrt broadcast via PE)
  - AllToAll reshards o_g head-split -> T-split (1MB/core vs 8.4MB
    AllGather); per-core o_proj over its 256-row T slice.
"""
import sys
if '/opt/trn_rl_repo' not in sys.path:
    sys.path.insert(0, '/opt/trn_rl_repo')
import numpy as np

import concourse.bacc as bacc
import concourse.mybir as mybir
import concourse.tile as tile
from concourse import bass_utils

F32 = mybir.dt.float32
F32R = mybir.dt.float32r
BF16 = mybir.dt.bfloat16
AF = mybir.ActivationFunctionType

H, DK, DV, M, T, D = 16, 128, 128, 64, 2048, 2048
EPS, CLAMP, ROPE_BASE = 1e-5, 32.0, 10000.0
N_CORES = 8
NT = T // 128        # 16
NB = T // 512        # 4 big row-chunks
ND = D // 128        # 16
SCALE = DK ** -0.5

REPEAT = 1
DEBUG = False


def build(repeat=1, debug=False):
    nc = bacc.Bacc(None, target_bir_lowering=False, debug=False, num_devices=N_CORES)

    din = {}
    for nm, shp, dt in [
        ("hsb", [128, NB, ND, 512], BF16),
        ("wq0", [128, ND, 128], BF16), ("wq1", [128, ND, 128], BF16),
        ("wk0", [128, ND, 128], BF16), ("wk1", [128, ND, 128], BF16),
        ("wg0", [128, ND, 128], BF16), ("wg1", [128, ND, 128], BF16),
        ("wvu", [128, ND, 272], BF16),
        ("ws2e", [17, 128], BF16),
        ("ones2k", [1, T], BF16),
        ("onesrow_b", [1, 128], BF16), ("onescol_b", [128, 1], BF16),
        ("onesrow_r", [1, 128], F32R), ("onescol_r", [128, 1], F32R),
        ("cossin", [128, T], F32),
        ("triu", [128, 128], BF16), ("ident", [128, 128], F32),
        ("masks", [128, 4, 512], BF16),
        ("woT", [128, ND, D], BF16),
    ]:
        din[nm] = nc.dram_tensor(nm, shp, dt, kind="ExternalInput")
    out_d = nc.dram_tensor("out", [256, D], F32, kind="ExternalOutput")

    dbg = {}
    if debug:
        for nm, shp, dt in [("qT", [256, T], BF16), ("kT", [256, T], BF16),
                            ("v", [128, NT * 256], BF16),
                            ("sg", [256, T], BF16), ("es", [128, NT * 128], BF16),
                            ("enzT", [128, T], F32), ("esT", [128, T], BF16),
                            ("u17", [17, T], BF16), ("qveT", [128, T], BF16),
                            ("ogT", [256, T], BF16)]:
            dbg[nm] = nc.dram_tensor("dbg_" + nm, shp, dt, kind="ExternalOutput")

    with tile.TileContext(nc) as tc:
        with tc.tile_pool(name="const", bufs=1) as cpool, \
             tc.tile_pool(name="big", bufs=1) as big:
            c = {}
            for nm in ("ws2e", "onesrow_b", "onescol_b", "onesrow_r",
                       "onescol_r", "triu", "ident"):
                tl = cpool.tile(list(din[nm].shape), din[nm].dtype, tag=nm, name=nm)
                nc.sync.dma_start(tl[:], din[nm].ap())
                c[nm] = tl

            st = {
                "qT": [big.tile([128, T], BF16, tag=f"qT{h}", name=f"qT{h}") for h in range(2)],
                "kT": [big.tile([128, T], BF16, tag=f"kT{h}", name=f"kT{h}") for h in range(2)],
                "sg": [big.tile([128, T], BF16, tag=f"sg{h}", name=f"sg{h}") for h in range(2)],
                "v": big.tile([128, NT, 256], BF16, tag="v", name="v"),
                "u17": big.tile([17, T], BF16, tag="u17", name="u17"),
                "es_t": big.tile([128, NT, 128], BF16, tag="es_t", name="es_t"),
                "esT": big.tile([128, T], BF16, tag="esT", name="esT"),
                "enzT": big.tile([128, T], F32, tag="enzT", name="enzT"),
            }
            for _ in range(repeat):
                _pass(nc, tc, din, c, st, out_d, dbg, debug)

    nc.compile()
    return nc, dbg


def _pass(nc, tc, din, c, st, out_d, dbg, debug):
    qT, kT, sg = st["qT"], st["kT"], st["sg"]
    v, u17, es_t, esT, enzT = st["v"], st["u17"], st["es_t"], st["esT"], st["enzT"]

    # ================= PHASE 1: projections (4 x 512-col sweeps) =================
    with tc.tile_pool(name="p1w", bufs=1) as p1w, \
         tc.tile_pool(name="p1sb", bufs=2) as p1sb, \
         tc.tile_pool(name="p1hs", bufs=2) as p1hs:
        p1ps_cm = tc.tile_pool(name="p1ps", bufs=1, space="PSUM")
        p1ps = p1ps_cm.__enter__()
        # hs chunk 0 first (4 sub-tiles of 4 d-groups each), then weights:
        # the first matmul needs only hsq sub 0 + wq0, so compute starts
        # ~6us in instead of waiting for the full weight set
        engs = [nc.sync, nc.scalar, nc.gpsimd]

        def hs_load(tb, spread=False):
            subs = []
            for s in range(4):
                t = p1hs.tile([128, 4, 512], BF16, tag=f"hsq{s}",
                              name=f"hsq{tb}_{s}")
                eng = engs[s % 3] if spread else nc.sync
                eng.dma_start(t[:], din["hsb"].ap()[:, tb, 4 * s:4 * s + 4])
                subs.append(t)
            return subs

        hs_next = hs_load(0, spread=True)
        wts = {}
        for i, wn in enumerate(("wq0", "wq1", "wk0", "wk1", "wg0", "wg1", "wvu")):
            cw = din[wn].shape[2]
            wt = p1w.tile([128, ND, cw], BF16, tag=wn, name=wn)
            engs[i % 3].dma_start(wt[:], din[wn].ap())
            wts[wn] = wt
        cossin = p1w.tile([128, T], F32, tag="cossin", name="cossin")
        nc.scalar.dma_start(cossin[:], din["cossin"].ap())
        nc.gpsimd.dma_start(u17[16:17, :], din["ones2k"].ap())

        carries = []
        QKG = (("wq0", qT[0], "q"), ("wq1", qT[1], "q"),
               ("wk0", kT[0], "k"), ("wk1", kT[1], "k"),
               ("wg0", sg[0], "g"), ("wg1", sg[1], "g"))
        for tb in range(NB):
            tsl = slice(tb * 512, (tb + 1) * 512)
            hsq_s = hs_next
            if tb + 1 < NB:
                hs_next = hs_load(tb + 1)
            acc = {wn: p1ps.tile([128, 512], F32, tag=f"acc_{wn}",
                                 name=f"acc_{wn}_{tb}")
                   for wn, *_ in QKG}
            # qkg d-loop first: these 6 banks are freed by fast RoPE/silu
            # drains, so the NEXT tb's qkg matmuls never stall; the slot
            # chain below lives on the v banks and overlaps that d-loop
            for d in range(ND):
                hs_d = hsq_s[d // 4][:, d % 4, :]
                for wn, dst, kind in QKG:
                    nc.tensor.matmul(acc[wn][:], wts[wn][:, d, :], hs_d,
                                     start=(d == 0), stop=(d == ND - 1))
            accv = [p1ps.tile([128, 272], F32, tag=f"acc_v{i}", name=f"acc_v{i}_{tb}")
                    for i in range(2)]
            for d in range(ND):
                hs_d = hsq_s[d // 4][:, d % 4, :]
                for i in range(2):
                    nc.tensor.matmul(accv[i][:],
                                     hs_d[:, i * 128:(i + 1) * 128],
                                     wts["wvu"][:, d, :],
                                     start=(d == 0), stop=(d == ND - 1))
            # drains: RoPE for q/k, silu for g
            for wn, dst, kind in QKG:
                ps = acc[wn]
                if kind == "g":
                    sgm = p1sb.tile([128, 512], F32, tag="sgm")
                    nc.scalar.activation(sgm[:], ps[:], AF.Sigmoid)
                    nc.vector.tensor_mul(dst[:, tsl], ps[:], sgm[:])
                else:
                    t1 = p1sb.tile([64, 512], F32, tag="ropet1")
                    t2 = p1sb.tile([64, 512], F32, tag="ropet2")
                    nc.vector.tensor_mul(t1[:], ps[0:64, :], cossin[0:64, tsl])
                    nc.vector.tensor_mul(t2[:], ps[64:128, :], cossin[64:128, tsl])
                    nc.vector.tensor_sub(dst[0:64, tsl], t1[:], t2[:])
                    nc.vector.tensor_mul(t1[:], ps[64:128, :], cossin[0:64, tsl])
                    nc.vector.tensor_mul(t2[:], ps[0:64, :], cossin[64:128, tsl])
                    nc.vector.tensor_add(dst[64:128, tsl], t1[:], t2[:])
            # v+u drains for subblocks 0,1; then second v+u pass for 2,3
            def drain_vu(i, accv_i):
                ts = 4 * tb + i
                nc.scalar.copy(v[:, ts, :], accv_i[:, 0:256])
                usb = p1sb.tile([128, 16], F32, tag="usb")
                nc.scalar.copy(usb[:], accv_i[:, 256:272])
                utp = p1ps.tile([16, 128], F32, tag="acc_v0", name=f"utp{ts}")
                nc.tensor.transpose(utp[:], usb[:], c["ident"][:])
                nc.scalar.copy(u17[0:16, ts * 128:(ts + 1) * 128], utp[:])

            drain_vu(0, accv[0])
            drain_vu(1, accv[1])
            accv2 = [p1ps.tile([128, 272], F32, tag=f"acc_v{i}", name=f"acc_v2{i}_{tb}")
                     for i in range(2)]
            for d in range(ND):
                hs_d = hsq_s[d // 4][:, d % 4, :]
                for i in range(2):
                    nc.tensor.matmul(accv2[i][:],
                                     hs_d[:, (2 + i) * 128:(3 + i) * 128],
                                     wts["wvu"][:, d, :],
                                     start=(d == 0), stop=(d == ND - 1))
            drain_vu(2, accv2[0])
            drain_vu(3, accv2[1])

            # slot logits for this chunk: esT (col-oriented) + es_t/cumsum
            pse = p1ps.tile([128, 512], F32, tag="acc_v0", name=f"psesT{tb}")
            nc.tensor.matmul(pse[:], c["ws2e"][:], u17[:, tsl], start=True, stop=True)
            nc.scalar.activation(esT[:, tsl], pse[:], AF.Exp)
            for ts in range(4 * tb, 4 * tb + 4):
                ssl = slice(ts * 128, (ts + 1) * 128)
                pss = p1ps.tile([128, 128], F32, tag="acc_v1", name=f"ps_st{ts}")
                nc.tensor.matmul(pss[:], u17[:, ssl], c["ws2e"][:],
                                 start=True, stop=True)
                nc.scalar.activation(es_t[:, ts, :], pss[:], AF.Exp)
                cs2 = p1ps.tile([128, 128], F32, tag="acc_v0", name=f"ps_cs{ts}")
                nc.tensor.matmul(cs2[:], c["triu"][:], es_t[:, ts, :],
                                 start=True, stop=(ts == 0))
                if ts > 0:
                    nc.tensor.matmul(cs2[:], c["onesrow_b"][:], carries[ts - 1][:],
                                     start=False, stop=True)
                enz_sb = p1sb.tile([128, 128], F32, tag="enz_sb")
                nc.vector.reciprocal_approx_fast(enz_sb[:], cs2[:])
                etp = p1ps.tile([128, 128], F32, tag="acc_v0", name=f"etp{ts}")
                nc.tensor.transpose(etp[:], enz_sb[:], c["ident"][:])
                nc.scalar.copy(enzT[:, ssl], etp[:])
                if ts < NT - 1:
                    csum = p1ps.tile([1, 128], F32, tag="acc_v1",
                                     name=f"csum{ts}")
                    nc.tensor.matmul(csum[:], c["onescol_b"][:], es_t[:, ts, :],
                                     start=True, stop=(ts == 0))
                    if ts > 0:
                        nc.tensor.matmul(csum[:], c["onesrow_b"][:, 0:1],
                                         carries[ts - 1][:], start=False, stop=True)
                    cr = p1sb.tile([1, 128], BF16, tag="carry", name=f"carry{ts}",
                                   bufs=4)
                    nc.vector.tensor_copy(cr[:], csum[:])
                    carries.append(cr)
        p1ps_cm.__exit__(None, None, None)

    if debug:
        for h in range(2):
            nc.sync.dma_start(dbg["qT"].ap()[h * 128:(h + 1) * 128, :], qT[h][:])
            nc.sync.dma_start(dbg["kT"].ap()[h * 128:(h + 1) * 128, :], kT[h][:])
            nc.sync.dma_start(dbg["sg"].ap()[h * 128:(h + 1) * 128, :], sg[h][:])
        nc.sync.dma_start(dbg["v"].ap(), v[:].rearrange("p a b -> p (a b)"))
        nc.sync.dma_start(dbg["es"].ap(), es_t[:].rearrange("p a b -> p (a b)"))
        nc.sync.dma_start(dbg["enzT"].ap(), enzT[:])
        nc.sync.dma_start(dbg["esT"].ap(), esT[:])
        nc.sync.dma_start(dbg["u17"].ap(), u17[:])

    # ================= PHASE 2: attention =================
    with tc.tile_pool(name="p2sb", bufs=3) as p2sb, \
         tc.tile_pool(name="qveP", bufs=2) as qvep, \
         tc.tile_pool(name="dramp", bufs=1, space="DRAM") as dpool:
        ag_in, ag_out = [], []
        for h in range(2):
            ag_in.append(dpool.tile([N_CORES, 128, 256], BF16,
                                    tag=f"ag_in{h}", name=f"ag_in{h}"))
            ag_out.append(dpool.tile([N_CORES, 128, 256], BF16,
                                     tag=f"ag_out{h}", name=f"ag_out{h}"))
        warm_in = dpool.tile([N_CORES, 16], BF16, tag="warm_in", name="warm_in")
        warm_out = dpool.tile([N_CORES, 16], BF16, tag="warm_out", name="warm_out")
        wsrc = p2sb.tile([1, 16 * N_CORES], BF16, tag="wsrc", bufs=1)
        nc.vector.memset(wsrc[:], 0.0)
        nc.sync.dma_start(warm_in[:].rearrange("s t -> (s t)"), wsrc[0])
        epsb = p2sb.tile([128, 1], F32, tag="epsb", bufs=1)
        nc.vector.memset(epsb[:], EPS)
        tl = p2sb.tile(list(din["masks"].shape), BF16, tag="masks", name="masks", bufs=1)
        nc.sync.dma_start(tl[:], din["masks"].ap())
        c["masks"] = tl
        # o_proj weights: prefetch the full 8MB during attention
        wo_sb = p2sb.tile([128, ND, D], BF16, tag="wo_sb", name="wo_sb", bufs=1)
        nc.sync.dma_start(wo_sb[:], din["woT"].ap())

        with tc.tile_pool(name="ps_at", bufs=3, space="PSUM") as ps_at, \
             tc.tile_pool(name="ps_ok", bufs=2, space="PSUM") as ps_ok, \
             tc.tile_pool(name="ps_ot", bufs=2, space="PSUM") as ps_ot, \
             tc.tile_pool(name="ps_aux", bufs=1, space="PSUM") as ps_aux:
            def stage1(I):
                rsl = slice(I * 512, (I + 1) * 512)
                njs = 4 * I + 4
                okT = ps_ok.tile([128, 512], F32, tag="okT", name=f"okT{I}",
                                 bufs=2)
                at_sb = {}

                def at_pair(j):
                    for h in range(2):
                        at = ps_at.tile([128, 512], F32, tag="atw2")
                        nc.tensor.matmul(at[:], kT[h][:, j * 128:(j + 1) * 128],
                                         qT[h][:, rsl], start=True, stop=True)
                        a_sb = p2sb.tile([128, 512], BF16, tag="at_sb")
                        m = j - 4 * I
                        if m >= 0:
                            nc.vector.tensor_mul(a_sb[:], at[:], c["masks"][:, m, :])
                        elif h == 0:
                            nc.scalar.copy(a_sb[:], at[:])
                        else:
                            nc.vector.tensor_copy(a_sb[:], at[:])
                        at_sb[(j, h)] = a_sb

                at_pair(0)
                for j in range(njs):
                    if j + 1 < njs:
                        at_pair(j + 1)
                    for h in range(2):
                        nc.tensor.matmul(okT[h * 64:(h + 1) * 64, :],
                                         es_t[:, j, h * 64:(h + 1) * 64],
                                         at_sb.pop((j, h))[:],
                                         start=(j == 0),
                                         stop=(j == njs - 1))
                return okT

            def softmax(I, okT):
                # unnormalized: qveT = exp(okT * enzT) * enzT  (RMS-norm
                # downstream cancels the per-(t,h) softmax denominator)
                rsl = slice(I * 512, (I + 1) * 512)
                qveT = qvep.tile([128, 512], BF16, tag="qveT")
                okm = p2sb.tile([128, 512], F32, tag="okm")
                nc.vector.tensor_mul(okm[:], okT[:], enzT[:, rsl])
                eok = p2sb.tile([128, 512], F32, tag="eok")
                nc.scalar.activation(eok[:], okm[:], AF.Exp)
                nc.vector.tensor_mul(qveT[:], eok[:], enzT[:, rsl])
                return qveT

            def stage2(I, qveT):
                rsl = slice(I * 512, (I + 1) * 512)
                njs = 4 * I + 4
                oT = [ps_ot.tile([128, 512], F32, tag="oT", name=f"oT{i}") for i in range(2)]
                w2_sb = {}

                def w2_pair(j):
                    for h in range(2):
                        w2 = ps_at.tile([128, 512], F32, tag="atw2")
                        nc.tensor.matmul(w2[:],
                                         esT[h * 64:(h + 1) * 64, j * 128:(j + 1) * 128],
                                         qveT[h * 64:(h + 1) * 64, :],
                                         start=True, stop=True)
                        wsb = p2sb.tile([128, 512], BF16, tag="at_sb")
                        m = j - 4 * I
                        if m >= 0:
                            nc.vector.tensor_mul(wsb[:], w2[:], c["masks"][:, m, :])
                        elif h == 0:
                            nc.scalar.copy(wsb[:], w2[:])
                        else:
                            nc.vector.tensor_copy(wsb[:], w2[:])
                        w2_sb[(j, h)] = wsb

                w2_pair(0)
                for j in range(njs):
                    if j + 1 < njs:
                        w2_pair(j + 1)
                    for h in range(2):
                        nc.tensor.matmul(oT[h][:], v[:, j, h * 128:(h + 1) * 128],
                                         w2_sb.pop((j, h))[:],
                                         start=(j == 0), stop=(j == njs - 1))
                # epilogue: o_g = o * rsqrt(mean o^2 + eps) * sg -> bf16 -> a2a_in
                for h in range(2):
                    sq = p2sb.tile([128, 512], F32R, tag="sq")
                    nc.scalar.activation(sq[:], oT[h][:], AF.Square)
                    ssq = ps_aux.tile([1, 512], F32, tag="aux")
                    nc.tensor.matmul(ssq[:], c["onescol_r"][:], sq[:], start=True, stop=True)
                    ssq_sb = p2sb.tile([1, 512], F32R, tag="ssq_sb")
                    with nc.allow_low_precision(reason="f32r bitcast for broadcast"):
                        nc.scalar.copy(ssq_sb[:], ssq[:])
                    rb = ps_aux.tile([128, 512], F32, tag="aux")
                    nc.tensor.matmul(rb[:], c["onesrow_r"][:], ssq_sb[:], start=True, stop=True)
                    rms = p2sb.tile([128, 512], F32, tag="rms")
                    nc.scalar.activation(rms[:], rb[:], AF.Sqrt, scale=1.0 / DV,
                                         bias=epsb[:])
                    rinv = p2sb.tile([128, 512], F32, tag="rinv")
                    nc.vector.reciprocal_approx_fast(rinv[:], rms[:])
                    t1 = p2sb.tile([128, 512], F32, tag="ept1")
                    nc.vector.tensor_mul(t1[:], oT[h][:], sg[h][:, rsl])
                    ogt = p2sb.tile([128, 512], BF16, tag="ogt")
                    nc.vector.tensor_mul(ogt[:], t1[:], rinv[:])
                    if debug:
                        nc.gpsimd.dma_start(dbg["ogT"].ap()[h * 128:(h + 1) * 128, rsl], ogt[:])
                    nc.sync.dma_start(ag_in[h][2 * I, :, :], ogt[:, 0:256])
                    nc.sync.dma_start(ag_in[h][2 * I + 1, :, :], ogt[:, 256:512])

            ok_prev = stage1(0)
            qv_prev = softmax(0, ok_prev)
            if debug:
                nc.sync.dma_start(dbg["qveT"].ap()[:, 0:512], qv_prev[:])
            for I in range(1, NB):
                okI = stage1(I)
                if I == 2:
                    # tiny warm-up A2A: keeps collective channels hot so the
                    # real transfers at the end run at steady-state latency
                    nc.gpsimd.collective_compute(
                        "AllToAll", mybir.AluOpType.bypass,
                        replica_groups=[list(range(N_CORES))],
                        ins=[warm_in[:].opt()], outs=[warm_out[:].opt()])
                qv_next = softmax(I, okI)
                stage2(I - 1, qv_prev)
                qv_prev = qv_next
                if debug:
                    nc.sync.dma_start(dbg["qveT"].ap()[:, I * 512:(I + 1) * 512], qv_prev[:])
            stage2(NB - 1, qv_prev)

        # ================= PHASE 3: per-head A2A + o_proj =================
        # A2A(h) starts once stage2(3)'s h epilogue DMAs land; o_proj h=0
        # matmuls overlap the h=1 transfer
        for h in range(2):
            nc.gpsimd.collective_compute(
                "AllToAll", mybir.AluOpType.bypass,
                replica_groups=[list(range(N_CORES))],
                ins=[ag_in[h][:].opt()], outs=[ag_out[h][:].opt()])
        og = {}
        for h in range(2):
            for s in range(N_CORES):
                ot = p2sb.tile([128, 256], BF16, tag=f"og{h}{s}",
                               name=f"og{h}{s}", bufs=1)
                nc.sync.dma_start(ot[:], ag_out[h][s, :, :])
                og[(h, s)] = ot
        p3ps_cm = tc.tile_pool(name="p3ps", bufs=1, space="PSUM")
        p3ps = p3ps_cm.__enter__()
        for th in range(2):
            pso = [p3ps.tile([128, 512], F32, tag=f"pso{th}{ns}",
                             name=f"pso{th}{ns}", bufs=1) for ns in range(4)]
            for n, (h, s) in enumerate((hh, ss) for hh in range(2)
                                       for ss in range(N_CORES)):
                kc = 2 * s + h
                for ns in range(4):
                    nc.tensor.matmul(pso[ns][:],
                                     og[(h, s)][:, th * 128:(th + 1) * 128],
                                     wo_sb[:, kc, ns * 512:(ns + 1) * 512],
                                     start=(n == 0), stop=(n == 15))
            for ns in range(4):
                osb = p2sb.tile([128, 512], F32, tag="osb")
                nc.scalar.copy(osb[:], pso[ns][:])
                nc.sync.dma_start(
                    out_d.ap()[th * 128:(th + 1) * 128, ns * 512:(ns + 1) * 512],
                    osb[:])
        p3ps_cm.__exit__(None, None, None)


# ======================= host side =======================

def _host_inputs(inputs):
    import ml_dtypes
    BF = ml_dtypes.bfloat16
    hs = np.ascontiguousarray(np.asarray(inputs["hidden_states"], np.float32)[0])
    Wq = np.asarray(inputs["Wq"], np.float32)
    Wk = np.asarray(inputs["Wk"], np.float32)
    Wv = np.asarray(inputs["Wv"], np.float32)
    Wg = np.asarray(inputs["Wg"], np.float32)
    Wo = np.asarray(inputs["Wo"], np.float32)
    Ws1 = np.asarray(inputs["Ws1"], np.float32)
    Ws2 = np.asarray(inputs["Ws2"], np.float32)
    bs2 = np.asarray(inputs["bs2"], np.float32)
    gnw = np.asarray(inputs["g_norm_weight"], np.float32)

    hsT = hs.T  # [D, T]
    # hsb: [p, chunk, k, t] with d = k*128 + p
    hsb = np.ascontiguousarray(
        hsT.reshape(ND, 128, NB, 512).transpose(1, 2, 0, 3)).astype(BF)
    pos = np.arange(T, dtype=np.float64)
    inv = 1.0 / (ROPE_BASE ** (np.arange(0, DK, 2, dtype=np.float64) / DK))
    ang = pos[:, None] * inv[None, :]
    cos = np.cos(ang).T.astype(np.float32)       # [64, T]
    sin = np.sin(ang).T.astype(np.float32)
    cossin = np.concatenate([cos, sin], axis=0).astype(np.float32)
    triu = np.triu(np.ones((128, 128), np.float32)).astype(BF)
    masks = np.zeros((128, 4, 512), np.float32)
    p = np.arange(128)[:, None]
    r = np.arange(512)[None, :]
    for m in range(4):
        masks[:, m, :] = (128 * m + p <= r).astype(np.float32)
    ident = np.eye(128, dtype=np.float32)
    onesrow = np.ones((1, 128), np.float32)
    ones2k = np.ones((1, T), np.float32).astype(BF)
    onescol = np.ones((128, 1), np.float32)
    # woT: [p, kc, n] with hd = kc*128 + p; gnw folded in
    woT = (Wo.T * np.tile(gnw, H)[:, None]).astype(BF)
    woT = np.ascontiguousarray(woT.reshape(ND, 128, D).transpose(1, 0, 2))

    def wlay(w):  # [2048, 128] -> [p, k, c] bf16
        return np.ascontiguousarray(
            w.reshape(ND, 128, -1).transpose(1, 0, 2)).astype(BF)

    in_maps = []
    for core in range(N_CORES):
        sl = slice(core * 256, (core + 1) * 256)
        ssl = slice(core * 128, (core + 1) * 128)
        ws2e = np.concatenate([Ws2[ssl].T, bs2[None, ssl]], axis=0).astype(BF)
        wvu = np.concatenate([Wv[sl].T, Ws1.T], axis=1)  # [2048, 272]
        m = {
            "hsb": hsb,
            "wq0": wlay(Wq[sl].T[:, 0:128] * SCALE),
            "wq1": wlay(Wq[sl].T[:, 128:256] * SCALE),
            "wk0": wlay(Wk[sl].T[:, 0:128]),
            "wk1": wlay(Wk[sl].T[:, 128:256]),
            "wg0": wlay(Wg[sl].T[:, 0:128]),
            "wg1": wlay(Wg[sl].T[:, 128:256]),
            "wvu": wlay(wvu),
            "ws2e": ws2e,
            "onesrow_b": onesrow.astype(BF), "onescol_b": onescol.astype(BF),
            "onesrow_r": onesrow, "onescol_r": onescol,
            "ones2k": ones2k,
            "cossin": cossin,
            "triu": triu, "masks": masks.astype(BF), "ident": ident,
            "woT": woT,
        }
        in_maps.append(m)
    return in_maps


_CACHE = {}


def kernel(**inputs):
    key = ("k", REPEAT, DEBUG)
    if key not in _CACHE:
        _CACHE[key] = build(repeat=REPEAT, debug=DEBUG)
    nc, dbg = _CACHE[key]
    in_maps = _host_inputs(inputs)
    res = bass_utils.run_bass_kernel_spmd(nc, in_maps, core_ids=list(range(N_CORES)))
    out = np.concatenate([res.results[c]["out"] for c in range(N_CORES)], axis=0)
    kernel.last_results = res
    return out.reshape(1, T, D).astype(np.float32)


# revision 24
# speedup vs baseline: 1.5674x; 1.0020x over previous
"""ABC attention (gated slot attention) on 8 TRN2 NeuronCores.

Sharding: 2 heads per core (16 heads / 8 cores). Per core:
  - projections q,k (RoPE, q pre-scaled), v, silu(gate), slot logits,
    all matmuls bf16 (2x stream rate vs f32r), moving dim 512
  - unnormalized softmax: RMS-norm downstream is scale-invariant, so
    softmax keeps only exp(ok*enz)*enz; enz applied in [m,t] layout
    (enzT) -> no per-row transposes/reductions/reciprocals
  - quadratic chunked ABC attention, causal masking via mask tiles
  - fused RMS-norm x gate epilogue (Rsqrt broadcast via PE)
  - AllToAll reshards o_g head-split -> T-split (1MB/core vs 8.4MB
    AllGather); per-core o_proj over its 256-row T slice.
"""
import sys
if '/opt/trn_rl_repo' not in sys.path:
    sys.path.insert(0, '/opt/trn_rl_repo')
import numpy as np

import concourse.bacc as bacc
import concourse.mybir as mybir
import concourse.tile as tile
from concourse import bass_utils

F32 = mybir.dt.float32
F32R = mybir.dt.float32r
BF16 = mybir.dt.bfloat16
AF = mybir.ActivationFunctionType

H, DK, DV, M, T, D = 16, 128, 128, 64, 2048, 2048
EPS, CLAMP, ROPE_BASE = 1e-5, 32.0, 10000.0
N_CORES = 8
NT = T // 128        # 16
NB = T // 512        # 4 big row-chunks
ND = D // 128        # 16
SCALE = DK ** -0.5

REPEAT = 1
DEBUG = False


def build(repeat=1, debug=False):
    nc = bacc.Bacc(None, target_bir_lowering=False, debug=False, num_devices=N_CORES)

    din = {}
    for nm, shp, dt in [
        ("hsb", [128, NB, ND, 512], BF16),
        ("wq0", [128, ND, 128], BF16), ("wq1", [128, ND, 128], BF16),
        ("wk0", [128, ND, 128], BF16), ("wk1", [128, ND, 128], BF16),
        ("wg0", [128, ND, 128], BF16), ("wg1", [128, ND, 128], BF16),
        ("wvu", [128, ND, 272], BF16),
        ("ws2e", [17, 128], BF16),
        ("ones2k", [1, T], BF16),
        ("onesrow_b", [1, 128], BF16), ("onescol_b", [128, 1], BF16),
        ("onesrow_r", [1, 128], F32R), ("onescol_r", [128, 1], F32R),
        ("cossin", [128, T], F32),
        ("triu", [128, 128], BF16), ("ident", [128, 128], F32),
        ("masks", [128, 4, 512], BF16),
        ("woT", [128, ND, D], BF16),
    ]:
        din[nm] = nc.dram_tensor(nm, shp, dt, kind="ExternalInput")
    out_d = nc.dram_tensor("out", [256, D], F32, kind="ExternalOutput")

    dbg = {}
    if debug:
        for nm, shp, dt in [("qT", [256, T], BF16), ("kT", [256, T], BF16),
                            ("v", [128, NT * 256], BF16),
                            ("sg", [256, T], BF16), ("es", [128, NT * 128], BF16),
                            ("enzT", [128, T], F32), ("esT", [128, T], BF16),
                            ("u17", [17, T], BF16), ("qveT", [128, T], BF16),
                            ("ogT", [256, T], BF16)]:
            dbg[nm] = nc.dram_tensor("dbg_" + nm, shp, dt, kind="ExternalOutput")

    with tile.TileContext(nc) as tc:
        with tc.tile_pool(name="const", bufs=1) as cpool, \
             tc.tile_pool(name="big", bufs=1) as big:
            c = {}
            for nm in ("ws2e", "onesrow_b", "onescol_b", "onesrow_r",
                       "onescol_r", "triu", "ident"):
                tl = cpool.tile(list(din[nm].shape), din[nm].dtype, tag=nm, name=nm)
                nc.sync.dma_start(tl[:], din[nm].ap())
                c[nm] = tl

            st = {
                "bigpool": big,
                "qT": [big.tile([128, T], BF16, tag=f"qT{h}", name=f"qT{h}") for h in range(2)],
                "kT": [big.tile([128, T], BF16, tag=f"kT{h}", name=f"kT{h}") for h in range(2)],
                "sg": [big.tile([128, T], BF16, tag=f"sg{h}", name=f"sg{h}") for h in range(2)],
                "v": big.tile([128, NT, 256], BF16, tag="v", name="v"),
                "u17": big.tile([17, T], BF16, tag="u17", name="u17"),
                "es_t": big.tile([128, NT, 128], BF16, tag="es_t", name="es_t"),
                "esT": big.tile([128, T], BF16, tag="esT", name="esT"),
                "enzT": big.tile([128, T], F32, tag="enzT", name="enzT"),
            }
            for _ in range(repeat):
                _pass(nc, tc, din, c, st, out_d, dbg, debug)

    nc.compile()
    return nc, dbg


def _pass(nc, tc, din, c, st, out_d, dbg, debug):
    qT, kT, sg = st["qT"], st["kT"], st["sg"]
    v, u17, es_t, esT, enzT = st["v"], st["u17"], st["es_t"], st["esT"], st["enzT"]

    # ================= PHASE 1: projections (4 x 512-col sweeps) =================
    with tc.tile_pool(name="p1w", bufs=1) as p1w, \
         tc.tile_pool(name="p1sb", bufs=2) as p1sb, \
         tc.tile_pool(name="p1hs", bufs=2) as p1hs:
        p1ps_cm = tc.tile_pool(name="p1ps", bufs=1, space="PSUM")
        p1ps = p1ps_cm.__enter__()
        # hs chunk 0 first (4 sub-tiles of 4 d-groups each), then weights:
        # the first matmul needs only hsq sub 0 + wq0, so compute starts
        # ~6us in instead of waiting for the full weight set
        engs = [nc.sync, nc.scalar, nc.gpsimd]

        def hs_load(tb, spread=False):
            subs = []
            for s in range(4):
                t = p1hs.tile([128, 4, 512], BF16, tag=f"hsq{s}",
                              name=f"hsq{tb}_{s}")
                eng = engs[s % 3] if spread else nc.sync
                eng.dma_start(t[:], din["hsb"].ap()[:, tb, 4 * s:4 * s + 4])
                subs.append(t)
            return subs

        hs_next = hs_load(0, spread=True)
        wts = {}
        for i, wn in enumerate(("wq0", "wq1", "wk0", "wk1", "wg0", "wg1", "wvu")):
            cw = din[wn].shape[2]
            wt = p1w.tile([128, ND, cw], BF16, tag=wn, name=wn)
            engs[i % 3].dma_start(wt[:], din[wn].ap())
            wts[wn] = wt
        cossin = p1w.tile([128, T], F32, tag="cossin", name="cossin")
        nc.scalar.dma_start(cossin[:], din["cossin"].ap())
        nc.gpsimd.dma_start(u17[16:17, :], din["ones2k"].ap())

        carries = []

        def mk_pt(pool, tags):
            state = {"i": 0}

            def pt(shape, name):
                t = pool.tile(shape, F32, tag=tags[state["i"] % len(tags)],
                              name=name)
                state["i"] += 1
                return t
            return pt

        def slot_chain(tb, pt, sbp):
            tsl = slice(tb * 512, (tb + 1) * 512)
            pse = pt([128, 512], f"psesT{tb}")
            nc.tensor.matmul(pse[:], c["ws2e"][:], u17[:, tsl], start=True, stop=True)
            nc.scalar.activation(esT[:, tsl], pse[:], AF.Exp)
            for ts in range(4 * tb, 4 * tb + 4):
                ssl = slice(ts * 128, (ts + 1) * 128)
                pss = pt([128, 128], f"ps_st{ts}")
                nc.tensor.matmul(pss[:], u17[:, ssl], c["ws2e"][:],
                                 start=True, stop=True)
                nc.scalar.activation(es_t[:, ts, :], pss[:], AF.Exp)
                cs2 = pt([128, 128], f"ps_cs{ts}")
                nc.tensor.matmul(cs2[:], c["triu"][:], es_t[:, ts, :],
                                 start=True, stop=(ts == 0))
                if ts > 0:
                    nc.tensor.matmul(cs2[:], c["onesrow_b"][:], carries[ts - 1][:],
                                     start=False, stop=True)
                enz_sb = sbp.tile([128, 128], F32, tag="enz_sb", name=f"enz{ts}")
                nc.vector.reciprocal_approx_fast(enz_sb[:], cs2[:])
                etp = pt([128, 128], f"etp{ts}")
                nc.tensor.transpose(etp[:], enz_sb[:], c["ident"][:])
                nc.scalar.copy(enzT[:, ssl], etp[:])
                if ts < NT - 1:
                    csum = pt([1, 128], f"csum{ts}")
                    nc.tensor.matmul(csum[:], c["onescol_b"][:], es_t[:, ts, :],
                                     start=True, stop=(ts == 0))
                    if ts > 0:
                        nc.tensor.matmul(csum[:], c["onesrow_b"][:, 0:1],
                                         carries[ts - 1][:], start=False, stop=True)
                    cr = st["bigpool"].tile([1, 128], BF16, tag=f"carry{ts}",
                                            name=f"carry{ts}")
                    nc.vector.tensor_copy(cr[:], csum[:])
                    carries.append(cr)
        st["slot_chain"] = slot_chain

        QKG = (("wq0", qT[0], "q"), ("wq1", qT[1], "q"),
               ("wk0", kT[0], "k"), ("wk1", kT[1], "k"),
               ("wg0", sg[0], "g"), ("wg1", sg[1], "g"))
        for tb in range(NB):
            tsl = slice(tb * 512, (tb + 1) * 512)
            hsq_s = hs_next
            if tb + 1 < NB:
                hs_next = hs_load(tb + 1)
            acc = {wn: p1ps.tile([128, 512], F32, tag=f"acc_{wn}",
                                 name=f"acc_{wn}_{tb}")
                   for wn, *_ in QKG}
            # qkg d-loop first: these 6 banks are freed by fast RoPE/silu
            # drains, so the NEXT tb's qkg matmuls never stall; the slot
            # chain below lives on the v banks and overlaps that d-loop
            for d in range(ND):
                hs_d = hsq_s[d // 4][:, d % 4, :]
                for wn, dst, kind in QKG:
                    nc.tensor.matmul(acc[wn][:], wts[wn][:, d, :], hs_d,
                                     start=(d == 0), stop=(d == ND - 1))
            accv = [p1ps.tile([128, 272], F32, tag=f"acc_v{i}", name=f"acc_v{i}_{tb}")
                    for i in range(2)]
            for d in range(ND):
                hs_d = hsq_s[d // 4][:, d % 4, :]
                for i in range(2):
                    nc.tensor.matmul(accv[i][:],
                                     hs_d[:, i * 128:(i + 1) * 128],
                                     wts["wvu"][:, d, :],
                                     start=(d == 0), stop=(d == ND - 1))
            # drains: RoPE for q/k, silu for g
            for wn, dst, kind in QKG:
                ps = acc[wn]
                if kind == "g":
                    sgm = p1sb.tile([128, 512], F32, tag="sgm")
                    nc.scalar.activation(sgm[:], ps[:], AF.Sigmoid)
                    nc.vector.tensor_mul(dst[:, tsl], ps[:], sgm[:])
                else:
                    t1 = p1sb.tile([64, 512], F32, tag="ropet1")
                    t2 = p1sb.tile([64, 512], F32, tag="ropet2")
                    nc.vector.tensor_mul(t1[:], ps[0:64, :], cossin[0:64, tsl])
                    nc.vector.tensor_mul(t2[:], ps[64:128, :], cossin[64:128, tsl])
                    nc.vector.tensor_sub(dst[0:64, tsl], t1[:], t2[:])
                    nc.vector.tensor_mul(t1[:], ps[64:128, :], cossin[0:64, tsl])
                    nc.vector.tensor_mul(t2[:], ps[0:64, :], cossin[64:128, tsl])
                    nc.vector.tensor_add(dst[64:128, tsl], t1[:], t2[:])
            # v+u drains for subblocks 0,1; then second v+u pass for 2,3
            def drain_vu(i, accv_i):
                ts = 4 * tb + i
                nc.scalar.copy(v[:, ts, :], accv_i[:, 0:256])
                usb = p1sb.tile([128, 16], F32, tag="usb")
                nc.scalar.copy(usb[:], accv_i[:, 256:272])
                utp = p1ps.tile([16, 128], F32, tag="acc_v0", name=f"utp{ts}")
                nc.tensor.transpose(utp[:], usb[:], c["ident"][:])
                nc.scalar.copy(u17[0:16, ts * 128:(ts + 1) * 128], utp[:])

            drain_vu(0, accv[0])
            drain_vu(1, accv[1])
            accv2 = [p1ps.tile([128, 272], F32, tag=f"acc_v{i}", name=f"acc_v2{i}_{tb}")
                     for i in range(2)]
            for d in range(ND):
                hs_d = hsq_s[d // 4][:, d % 4, :]
                for i in range(2):
                    nc.tensor.matmul(accv2[i][:],
                                     hs_d[:, (2 + i) * 128:(3 + i) * 128],
                                     wts["wvu"][:, d, :],
                                     start=(d == 0), stop=(d == ND - 1))
            drain_vu(2, accv2[0])
            drain_vu(3, accv2[1])

            if tb < NB - 1:
                # slot logits (esT, es_t, cumsum->enzT); overlaps next tb's
                # qkg d-loop since it lives on the freed v banks
                st["slot_chain"](tb, mk_pt(p1ps, ("acc_v0", "acc_v1")), p1sb)
        p1ps_cm.__exit__(None, None, None)

    if debug:
        for h in range(2):
            nc.sync.dma_start(dbg["qT"].ap()[h * 128:(h + 1) * 128, :], qT[h][:])
            nc.sync.dma_start(dbg["kT"].ap()[h * 128:(h + 1) * 128, :], kT[h][:])
            nc.sync.dma_start(dbg["sg"].ap()[h * 128:(h + 1) * 128, :], sg[h][:])
        nc.sync.dma_start(dbg["v"].ap(), v[:].rearrange("p a b -> p (a b)"))
        nc.sync.dma_start(dbg["es"].ap(), es_t[:].rearrange("p a b -> p (a b)"))
        nc.sync.dma_start(dbg["enzT"].ap(), enzT[:])
        nc.sync.dma_start(dbg["esT"].ap(), esT[:])
        nc.sync.dma_start(dbg["u17"].ap(), u17[:])

    # ================= PHASE 2: attention =================
    with tc.tile_pool(name="p2sb", bufs=3) as p2sb, \
         tc.tile_pool(name="qveP", bufs=2) as qvep, \
         tc.tile_pool(name="dramp", bufs=1, space="DRAM") as dpool:
        ag_in, ag_out = [], []
        for h in range(2):
            ag_in.append(dpool.tile([N_CORES, 128, 256], BF16,
                                    tag=f"ag_in{h}", name=f"ag_in{h}"))
            ag_out.append(dpool.tile([N_CORES, 128, 256], BF16,
                                     tag=f"ag_out{h}", name=f"ag_out{h}"))
        warm_in = dpool.tile([N_CORES, 16], BF16, tag="warm_in", name="warm_in")
        warm_out = dpool.tile([N_CORES, 16], BF16, tag="warm_out", name="warm_out")
        wsrc = p2sb.tile([1, 16 * N_CORES], BF16, tag="wsrc", bufs=1)
        nc.vector.memset(wsrc[:], 0.0)
        nc.sync.dma_start(warm_in[:].rearrange("s t -> (s t)"), wsrc[0])
        epsb = p2sb.tile([128, 1], F32, tag="epsb", bufs=1)
        nc.vector.memset(epsb[:], EPS)
        tl = p2sb.tile(list(din["masks"].shape), BF16, tag="masks", name="masks", bufs=1)
        nc.sync.dma_start(tl[:], din["masks"].ap())
        c["masks"] = tl
        # o_proj weights: prefetch the full 8MB during attention
        wo_sb = p2sb.tile([128, ND, D], BF16, tag="wo_sb", name="wo_sb", bufs=1)
        nc.sync.dma_start(wo_sb[:], din["woT"].ap())

        with tc.tile_pool(name="ps_at", bufs=3, space="PSUM") as ps_at, \
             tc.tile_pool(name="ps_ok", bufs=2, space="PSUM") as ps_ok, \
             tc.tile_pool(name="ps_ot", bufs=2, space="PSUM") as ps_ot, \
             tc.tile_pool(name="ps_aux", bufs=1, space="PSUM") as ps_aux:
            def mk_pt2(pool):
                def pt(shape, name):
                    return pool.tile(shape, F32, tag="oT", name=name)
                return pt

            def stage1(I):
                rsl = slice(I * 512, (I + 1) * 512)
                njs = 4 * I + 4
                okT = ps_ok.tile([128, 512], F32, tag="okT", name=f"okT{I}",
                                 bufs=2)
                at_sb = {}

                def at_pair(j):
                    for h in range(2):
                        at = ps_at.tile([128, 512], F32, tag="atw2")
                        nc.tensor.matmul(at[:], kT[h][:, j * 128:(j + 1) * 128],
                                         qT[h][:, rsl], start=True, stop=True)
                        a_sb = p2sb.tile([128, 512], BF16, tag="at_sb")
                        m = j - 4 * I
                        if m >= 0:
                            nc.vector.tensor_mul(a_sb[:], at[:], c["masks"][:, m, :])
                        elif h == 0:
                            nc.scalar.copy(a_sb[:], at[:])
                        else:
                            nc.vector.tensor_copy(a_sb[:], at[:])
                        at_sb[(j, h)] = a_sb

                at_pair(0)
                for j in range(njs):
                    if j + 1 < njs:
                        at_pair(j + 1)
                    for h in range(2):
                        nc.tensor.matmul(okT[h * 64:(h + 1) * 64, :],
                                         es_t[:, j, h * 64:(h + 1) * 64],
                                         at_sb.pop((j, h))[:],
                                         start=(j == 0),
                                         stop=(j == njs - 1))
                return okT

            def softmax(I, okT):
                # unnormalized: qveT = exp(okT * enzT) * enzT  (RMS-norm
                # downstream cancels the per-(t,h) softmax denominator)
                rsl = slice(I * 512, (I + 1) * 512)
                qveT = qvep.tile([128, 512], BF16, tag="qveT")
                okm = p2sb.tile([128, 512], F32, tag="okm")
                nc.vector.tensor_mul(okm[:], okT[:], enzT[:, rsl])
                eok = p2sb.tile([128, 512], F32, tag="eok")
                nc.scalar.activation(eok[:], okm[:], AF.Exp)
                nc.vector.tensor_mul(qveT[:], eok[:], enzT[:, rsl])
                return qveT

            def stage2(I, qveT):
                rsl = slice(I * 512, (I + 1) * 512)
                njs = 4 * I + 4
                oT = [ps_ot.tile([128, 512], F32, tag="oT", name=f"oT{i}") for i in range(2)]
                w2_sb = {}

                def w2_pair(j):
                    for h in range(2):
                        w2 = ps_at.tile([128, 512], F32, tag="atw2")
                        nc.tensor.matmul(w2[:],
                                         esT[h * 64:(h + 1) * 64, j * 128:(j + 1) * 128],
                                         qveT[h * 64:(h + 1) * 64, :],
                                         start=True, stop=True)
                        wsb = p2sb.tile([128, 512], BF16, tag="at_sb")
                        m = j - 4 * I
                        if m >= 0:
                            nc.vector.tensor_mul(wsb[:], w2[:], c["masks"][:, m, :])
                        elif h == 0:
                            nc.scalar.copy(wsb[:], w2[:])
                        else:
                            nc.vector.tensor_copy(wsb[:], w2[:])
                        w2_sb[(j, h)] = wsb

                w2_pair(0)
                for j in range(njs):
                    if j + 1 < njs:
                        w2_pair(j + 1)
                    for h in range(2):
                        nc.tensor.matmul(oT[h][:], v[:, j, h * 128:(h + 1) * 128],
                                         w2_sb.pop((j, h))[:],
                                         start=(j == 0), stop=(j == njs - 1))
                # epilogue: o_g = o * rsqrt(mean o^2 + eps) * sg -> bf16 -> a2a_in
                for h in range(2):
                    sq = p2sb.tile([128, 512], F32R, tag="sq")
                    nc.scalar.activation(sq[:], oT[h][:], AF.Square)
                    ssq = ps_aux.tile([1, 512], F32, tag="aux")
                    nc.tensor.matmul(ssq[:], c["onescol_r"][:], sq[:], start=True, stop=True)
                    ssq_sb = p2sb.tile([1, 512], F32R, tag="ssq_sb")
                    with nc.allow_low_precision(reason="f32r bitcast for broadcast"):
                        nc.scalar.copy(ssq_sb[:], ssq[:])
                    rb = ps_aux.tile([128, 512], F32, tag="aux")
                    nc.tensor.matmul(rb[:], c["onesrow_r"][:], ssq_sb[:], start=True, stop=True)
                    rms = p2sb.tile([128, 512], F32, tag="rms")
                    nc.scalar.activation(rms[:], rb[:], AF.Sqrt, scale=1.0 / DV,
                                         bias=epsb[:])
                    rinv = p2sb.tile([128, 512], F32, tag="rinv")
                    nc.vector.reciprocal_approx_fast(rinv[:], rms[:])
                    t1 = p2sb.tile([128, 512], F32, tag="ept1")
                    nc.vector.tensor_mul(t1[:], oT[h][:], sg[h][:, rsl])
                    ogt = p2sb.tile([128, 512], BF16, tag="ogt")
                    nc.vector.tensor_mul(ogt[:], t1[:], rinv[:])
                    if debug:
                        nc.gpsimd.dma_start(dbg["ogT"].ap()[h * 128:(h + 1) * 128, rsl], ogt[:])
                    nc.sync.dma_start(ag_in[h][2 * I, :, :], ogt[:, 0:256])
                    nc.sync.dma_start(ag_in[h][2 * I + 1, :, :], ogt[:, 256:512])

            ok_prev = stage1(0)
            st["slot_chain"](NB - 1, mk_pt2(ps_ot), p2sb)
            qv_prev = softmax(0, ok_prev)
            if debug:
                nc.sync.dma_start(dbg["qveT"].ap()[:, 0:512], qv_prev[:])
            for I in range(1, NB):
                okI = stage1(I)
                if I == 2:
                    # tiny warm-up A2A: keeps collective channels hot so the
                    # real transfers at the end run at steady-state latency
                    nc.gpsimd.collective_compute(
                        "AllToAll", mybir.AluOpType.bypass,
                        replica_groups=[list(range(N_CORES))],
                        ins=[warm_in[:].opt()], outs=[warm_out[:].opt()])
                qv_next = softmax(I, okI)
                stage2(I - 1, qv_prev)
                qv_prev = qv_next
                if debug:
                    nc.sync.dma_start(dbg["qveT"].ap()[:, I * 512:(I + 1) * 512], qv_prev[:])
            stage2(NB - 1, qv_prev)

        # ================= PHASE 3: per-head A2A + o_proj =================
        # A2A(h) starts once stage2(3)'s h epilogue DMAs land; o_proj h=0
        # matmuls overlap the h=1 transfer
        for h in range(2):
            nc.gpsimd.collective_compute(
                "AllToAll", mybir.AluOpType.bypass,
                replica_groups=[list(range(N_CORES))],
                ins=[ag_in[h][:].opt()], outs=[ag_out[h][:].opt()])
        og = {}
        for h in range(2):
            for s in range(N_CORES):
                ot = p2sb.tile([128, 256], BF16, tag=f"og{h}{s}",
                               name=f"og{h}{s}", bufs=1)
                nc.sync.dma_start(ot[:], ag_out[h][s, :, :])
                og[(h, s)] = ot
        p3ps_cm = tc.tile_pool(name="p3ps", bufs=1, space="PSUM")
        p3ps = p3ps_cm.__enter__()
        for th in range(2):
            pso = [p3ps.tile([128, 512], F32, tag=f"pso{th}{ns}",
                             name=f"pso{th}{ns}", bufs=1) for ns in range(4)]
            for n, (h, s) in enumerate((hh, ss) for hh in range(2)
                                       for ss in range(N_CORES)):
                kc = 2 * s + h
                for ns in range(4):
                    nc.tensor.matmul(pso[ns][:],
                                     og[(h, s)][:, th * 128:(th + 1) * 128],
                                     wo_sb[:, kc, ns * 512:(ns + 1) * 512],
                                     start=(n == 0), stop=(n == 15))
            for ns in range(4):
                osb = p2sb.tile([128, 512], F32, tag="osb")
                nc.scalar.copy(osb[:], pso[ns][:])
                nc.sync.dma_start(
                    out_d.ap()[th * 128:(th + 1) * 128, ns * 512:(ns + 1) * 512],
                    osb[:])
        p3ps_cm.__exit__(None, None, None)


# ======================= host side =======================

def _host_inputs(inputs):
    import ml_dtypes
    BF = ml_dtypes.bfloat16
    hs = np.ascontiguousarray(np.asarray(inputs["hidden_states"], np.float32)[0])
    Wq = np.asarray(inputs["Wq"], np.float32)
    Wk = np.asarray(inputs["Wk"], np.float32)
    Wv = np.asarray(inputs["Wv"], np.float32)
    Wg = np.asarray(inputs["Wg"], np.float32)
    Wo = np.asarray(inputs["Wo"], np.float32)
    Ws1 = np.asarray(inputs["Ws1"], np.float32)
    Ws2 = np.asarray(inputs["Ws2"], np.float32)
    bs2 = np.asarray(inputs["bs2"], np.float32)
    gnw = np.asarray(inputs["g_norm_weight"], np.float32)

    hsT = hs.T  # [D, T]
    # hsb: [p, chunk, k, t] with d = k*128 + p
    hsb = np.ascontiguousarray(
        hsT.reshape(ND, 128, NB, 512).transpose(1, 2, 0, 3)).astype(BF)
    pos = np.arange(T, dtype=np.float64)
    inv = 1.0 / (ROPE_BASE ** (np.arange(0, DK, 2, dtype=np.float64) / DK))
    ang = pos[:, None] * inv[None, :]
    cos = np.cos(ang).T.astype(np.float32)       # [64, T]
    sin = np.sin(ang).T.astype(np.float32)
    cossin = np.concatenate([cos, sin], axis=0).astype(np.float32)
    triu = np.triu(np.ones((128, 128), np.float32)).astype(BF)
    masks = np.zeros((128, 4, 512), np.float32)
    p = np.arange(128)[:, None]
    r = np.arange(512)[None, :]
    for m in range(4):
        masks[:, m, :] = (128 * m + p <= r).astype(np.float32)
    ident = np.eye(128, dtype=np.float32)
    onesrow = np.ones((1, 128), np.float32)
    ones2k = np.ones((1, T), np.float32).astype(BF)
    onescol = np.ones((128, 1), np.float32)
    # woT: [p, kc, n] with hd = kc*128 + p; gnw folded in
    woT = (Wo.T * np.tile(gnw, H)[:, None]).astype(BF)
    woT = np.ascontiguousarray(woT.reshape(ND, 128, D).transpose(1, 0, 2))

    def wlay(w):  # [2048, 128] -> [p, k, c] bf16
        return np.ascontiguousarray(
            w.reshape(ND, 128, -1).transpose(1, 0, 2)).astype(BF)

    in_maps = []
    for core in range(N_CORES):
        sl = slice(core * 256, (core + 1) * 256)
        ssl = slice(core * 128, (core + 1) * 128)
        ws2e = np.concatenate([Ws2[ssl].T, bs2[None, ssl]], axis=0).astype(BF)
        wvu = np.concatenate([Wv[sl].T, Ws1.T], axis=1)  # [2048, 272]
        m = {
            "hsb": hsb,
            "wq0": wlay(Wq[sl].T[:, 0:128] * SCALE),
            "wq1": wlay(Wq[sl].T[:, 128:256] * SCALE),
            "wk0": wlay(Wk[sl].T[:, 0:128]),
            "wk1": wlay(Wk[sl].T[:, 128:256]),
            "wg0": wlay(Wg[sl].T[:, 0:128]),
            "wg1": wlay(Wg[sl].T[:, 128:256]),
            "wvu": wlay(wvu),
            "ws2e": ws2e,
            "onesrow_b": onesrow.astype(BF), "onescol_b": onescol.astype(BF),
            "onesrow_r": onesrow, "onescol_r": onescol,
            "ones2k": ones2k,
            "cossin": cossin,
            "triu": triu, "masks": masks.astype(BF), "ident": ident,
            "woT": woT,
        }
        in_maps.append(m)
    return in_maps


_CACHE = {}


def kernel(**inputs):
    key = ("k", REPEAT, DEBUG)
    if key not in _CACHE:
        _CACHE[key] = build(repeat=REPEAT, debug=DEBUG)
    nc, dbg = _CACHE[key]
    in_maps = _host_inputs(inputs)
    res = bass_utils.run_bass_kernel_spmd(nc, in_maps, core_ids=list(range(N_CORES)))
    out = np.concatenate([res.results[c]["out"] for c in range(N_CORES)], axis=0)
    kernel.last_results = res
    return out.reshape(1, T, D).astype(np.float32)


# revision 27
# speedup vs baseline: 1.5721x; 1.0030x over previous
"""ABC attention (gated slot attention) on 8 TRN2 NeuronCores.

Sharding: 2 heads per core (16 heads / 8 cores). Per core:
  - projections q,k (RoPE, q pre-scaled), v, silu(gate), slot logits,
    all matmuls bf16 (2x stream rate vs f32r), moving dim 512
  - unnormalized softmax: RMS-norm downstream is scale-invariant, so
    softmax keeps only exp(ok*enz)*enz; enz applied in [m,t] layout
    (enzT) -> no per-row transposes/reductions/reciprocals
  - quadratic chunked ABC attention, causal masking via mask tiles
  - fused RMS-norm x gate epilogue (Rsqrt broadcast via PE)
  - AllToAll reshards o_g head-split -> T-split (1MB/core vs 8.4MB
    AllGather); per-core o_proj over its 256-row T slice.
"""
import sys
if '/opt/trn_rl_repo' not in sys.path:
    sys.path.insert(0, '/opt/trn_rl_repo')
import numpy as np

import concourse.bacc as bacc
import concourse.mybir as mybir
import concourse.tile as tile
from concourse import bass_utils

F32 = mybir.dt.float32
F32R = mybir.dt.float32r
BF16 = mybir.dt.bfloat16
AF = mybir.ActivationFunctionType

H, DK, DV, M, T, D = 16, 128, 128, 64, 2048, 2048
EPS, CLAMP, ROPE_BASE = 1e-5, 32.0, 10000.0
N_CORES = 8
NT = T // 128        # 16
NB = T // 512        # 4 big row-chunks
ND = D // 128        # 16
SCALE = DK ** -0.5

REPEAT = 1
DEBUG = False


def build(repeat=1, debug=False):
    nc = bacc.Bacc(None, target_bir_lowering=False, debug=False, num_devices=N_CORES)

    din = {}
    for nm, shp, dt in [
        ("hsb", [128, NB, ND, 512], BF16),
        ("wq0", [128, ND, 128], BF16), ("wq1", [128, ND, 128], BF16),
        ("wk0", [128, ND, 128], BF16), ("wk1", [128, ND, 128], BF16),
        ("wg0", [128, ND, 128], BF16), ("wg1", [128, ND, 128], BF16),
        ("wvu", [128, ND, 272], BF16),
        ("ws2e", [17, 128], BF16),
        ("ones2k", [1, T], BF16),
        ("onesrow_b", [1, 128], BF16), ("onescol_b", [128, 1], BF16),
        ("onesrow_r", [1, 128], F32R), ("onescol_r", [128, 1], F32R),
        ("cossin", [128, T], F32),
        ("triu", [128, 128], BF16), ("ident", [128, 128], F32),
        ("masks", [128, 4, 512], BF16),
        ("woT", [128, ND, D], BF16),
    ]:
        din[nm] = nc.dram_tensor(nm, shp, dt, kind="ExternalInput")
    out_d = nc.dram_tensor("out", [256, D], F32, kind="ExternalOutput")

    dbg = {}
    if debug:
        for nm, shp, dt in [("qT", [256, T], BF16), ("kT", [256, T], BF16),
                            ("v", [128, NT * 256], BF16),
                            ("sg", [256, T], BF16), ("es", [128, NT * 128], BF16),
                            ("enzT", [128, T], F32), ("esT", [128, T], BF16),
                            ("u17", [17, T], BF16), ("qveT", [128, T], BF16),
                            ("ogT", [256, T], BF16)]:
            dbg[nm] = nc.dram_tensor("dbg_" + nm, shp, dt, kind="ExternalOutput")

    with tile.TileContext(nc) as tc:
        with tc.tile_pool(name="const", bufs=1) as cpool, \
             tc.tile_pool(name="big", bufs=1) as big:
            c = {}
            for nm in ("ws2e", "onesrow_b", "onescol_b", "onesrow_r",
                       "onescol_r", "triu", "ident"):
                tl = cpool.tile(list(din[nm].shape), din[nm].dtype, tag=nm, name=nm)
                nc.sync.dma_start(tl[:], din[nm].ap())
                c[nm] = tl

            st = {
                "bigpool": big,
                "qT": [big.tile([128, T], BF16, tag=f"qT{h}", name=f"qT{h}") for h in range(2)],
                "kT": [big.tile([128, T], BF16, tag=f"kT{h}", name=f"kT{h}") for h in range(2)],
                "sg": [big.tile([128, T], BF16, tag=f"sg{h}", name=f"sg{h}") for h in range(2)],
                "v": big.tile([128, NT, 256], BF16, tag="v", name="v"),
                "u17": big.tile([17, T], BF16, tag="u17", name="u17"),
                "es_t": big.tile([128, NT, 128], BF16, tag="es_t", name="es_t"),
                "esT": big.tile([128, T], BF16, tag="esT", name="esT"),
                "enzT": big.tile([128, T], F32, tag="enzT", name="enzT"),
            }
            for _ in range(repeat):
                _pass(nc, tc, din, c, st, out_d, dbg, debug)

    nc.compile()
    return nc, dbg


def _pass(nc, tc, din, c, st, out_d, dbg, debug):
    qT, kT, sg = st["qT"], st["kT"], st["sg"]
    v, u17, es_t, esT, enzT = st["v"], st["u17"], st["es_t"], st["esT"], st["enzT"]

    # ================= PHASE 1: projections (4 x 512-col sweeps) =================
    with tc.tile_pool(name="p1w", bufs=1) as p1w, \
         tc.tile_pool(name="p1sb", bufs=2) as p1sb, \
         tc.tile_pool(name="p1hs", bufs=2) as p1hs:
        p1ps_cm = tc.tile_pool(name="p1ps", bufs=1, space="PSUM")
        p1ps = p1ps_cm.__enter__()
        # hs chunk 0 first (4 sub-tiles of 4 d-groups each), then weights:
        # the first matmul needs only hsq sub 0 + wq0, so compute starts
        # ~6us in instead of waiting for the full weight set
        engs = [nc.sync, nc.scalar, nc.gpsimd]

        def hs_load(tb, spread=False):
            subs = []
            for s in range(4):
                t = p1hs.tile([128, 4, 512], BF16, tag=f"hsq{s}",
                              name=f"hsq{tb}_{s}")
                eng = engs[s % 3] if spread else nc.sync
                eng.dma_start(t[:], din["hsb"].ap()[:, tb, 4 * s:4 * s + 4])
                subs.append(t)
            return subs

        hs_next = [hs_load(0, spread=True), hs_load(1, spread=False)]
        wts = {}
        for i, wn in enumerate(("wq0", "wq1", "wk0", "wk1", "wg0", "wg1", "wvu")):
            cw = din[wn].shape[2]
            wt = p1w.tile([128, ND, cw], BF16, tag=wn, name=wn)
            engs[i % 3].dma_start(wt[:], din[wn].ap())
            wts[wn] = wt
        cossin = p1w.tile([128, T], F32, tag="cossin", name="cossin")
        nc.scalar.dma_start(cossin[:], din["cossin"].ap())
        nc.gpsimd.dma_start(u17[16:17, :], din["ones2k"].ap())

        carries = []

        def mk_pt(pool, tags):
            state = {"i": 0}

            def pt(shape, name):
                t = pool.tile(shape, F32, tag=tags[state["i"] % len(tags)],
                              name=name)
                state["i"] += 1
                return t
            return pt

        def slot_chain(tb, pt, sbp):
            # staged: pse/esT -> pssq/es_t (one wide exp) -> csum chain ->
            # cs2 quad -> one wide recip -> transpose quad -> one enzT copy
            tsl = slice(tb * 512, (tb + 1) * 512)
            t0 = 4 * tb
            pse = pt([128, 512], f"psesT{tb}")
            nc.tensor.matmul(pse[:], c["ws2e"][:], u17[:, tsl], start=True, stop=True)
            nc.scalar.activation(esT[:, tsl], pse[:], AF.Exp)
            pssq = pt([128, 4, 128], f"ps_st{tb}")
            for i in range(4):
                nc.tensor.matmul(pssq[:, i, :],
                                 u17[:, (t0 + i) * 128:(t0 + i + 1) * 128],
                                 c["ws2e"][:], start=True, stop=True)
            nc.scalar.activation(
                es_t[:, t0:t0 + 4, :].rearrange("p a b -> p (a b)"),
                pssq[:].rearrange("p a b -> p (a b)"), AF.Exp)
            for i in range(4):
                ts = t0 + i
                if ts >= NT - 1:
                    break
                csum = pt([1, 128], f"csum{ts}")
                nc.tensor.matmul(csum[:], c["onescol_b"][:], es_t[:, ts, :],
                                 start=True, stop=(ts == 0))
                if ts > 0:
                    nc.tensor.matmul(csum[:], c["onesrow_b"][:, 0:1],
                                     carries[ts - 1][:], start=False, stop=True)
                cr = st["bigpool"].tile([1, 128], BF16, tag=f"carry{ts}",
                                        name=f"carry{ts}")
                nc.vector.tensor_copy(cr[:], csum[:])
                carries.append(cr)
            cs2q = pt([128, 4, 128], f"ps_cs{tb}")
            for i in range(4):
                ts = t0 + i
                nc.tensor.matmul(cs2q[:, i, :], c["triu"][:], es_t[:, ts, :],
                                 start=True, stop=(ts == 0))
                if ts > 0:
                    nc.tensor.matmul(cs2q[:, i, :], c["onesrow_b"][:],
                                     carries[ts - 1][:], start=False, stop=True)
            enz_sb = sbp.tile([128, 512], F32, tag="enz_sb", name=f"enz{tb}")
            nc.vector.reciprocal_approx_fast(
                enz_sb[:], cs2q[:].rearrange("p a b -> p (a b)"))
            etpq = pt([128, 4, 128], f"etp{tb}")
            for i in range(4):
                nc.tensor.transpose(etpq[:, i, :],
                                    enz_sb[:, i * 128:(i + 1) * 128],
                                    c["ident"][:])
            nc.scalar.copy(enzT[:, tsl], etpq[:].rearrange("p a b -> p (a b)"))
        st["slot_chain"] = slot_chain
        st["mk_pt"] = mk_pt

        QKG = {"wq0": (qT[0], "q"), "wq1": (qT[1], "q"),
               "wk0": (kT[0], "k"), "wk1": (kT[1], "k"),
               "wg0": (sg[0], "g"), "wg1": (sg[1], "g")}
        GROUPS = (("wq0", "wq1"), ("wk0", "wk1"), ("wg0", "wg1"))
        TAGSETS = (("pa0", "pa1", "pa2", "pa3"), ("pb0", "pb1", "pb2", "pb3"))

        def drain_qkg(wn, ti, ps):
            # drains: RoPE for q/k (q pre-scaled), silu for g
            dst, kind = QKG[wn]
            tsl = slice(ti * 512, (ti + 1) * 512)
            if kind == "g":
                nc.scalar.activation(dst[:, tsl], ps[:], AF.Silu)
            else:
                t1 = p1sb.tile([64, 512], F32, tag="ropet1")
                t2 = p1sb.tile([64, 512], F32, tag="ropet2")
                nc.vector.tensor_mul(t1[:], ps[0:64, :], cossin[0:64, tsl])
                nc.vector.tensor_mul(t2[:], ps[64:128, :], cossin[64:128, tsl])
                nc.vector.tensor_sub(dst[0:64, tsl], t1[:], t2[:])
                nc.vector.tensor_mul(t1[:], ps[64:128, :], cossin[0:64, tsl])
                nc.vector.tensor_mul(t2[:], ps[0:64, :], cossin[64:128, tsl])
                nc.vector.tensor_add(dst[64:128, tsl], t1[:], t2[:])

        def drain_vu(ts, accv_i):
            nc.scalar.copy(v[:, ts, :], accv_i[:, 0:256])
            usb = p1sb.tile([128, 16], F32, tag="usb")
            nc.scalar.copy(usb[:], accv_i[:, 256:272])
            utp = p1ps.tile([16, 128], F32, tag="pb2", name=f"utp{ts}")
            nc.tensor.transpose(utp[:], usb[:], c["ident"][:])
            nc.scalar.copy(u17[0:16, ts * 128:(ts + 1) * 128], utp[:])

        # paired 512-chunks: each weight tile loads once per PAIR of chunks
        # (consecutive same-stationary matmuls run at stream rate)
        for pair in range(2):
            tbs = (2 * pair, 2 * pair + 1)
            hsq_p = hs_next
            if pair == 0:
                hs_next = [hs_load(2, spread=False), hs_load(3, spread=False)]

            def hs_d(ti, d):
                return hsq_p[ti][d // 4][:, d % 4, :]

            for gi, grp in enumerate(GROUPS):
                tags = TAGSETS[(2 * pair + gi) % 2]
                acc = {}
                for wi, wn in enumerate(grp):
                    for ti in range(2):
                        acc[(wn, ti)] = p1ps.tile(
                            [128, 512], F32, tag=tags[2 * wi + ti],
                            name=f"acc_{wn}_{ti}_p{pair}")
                for d in range(ND):
                    for wn in grp:
                        for ti in range(2):
                            nc.tensor.matmul(acc[(wn, ti)][:], wts[wn][:, d, :],
                                             hs_d(ti, d),
                                             start=(d == 0), stop=(d == ND - 1))
                for wn in grp:
                    for ti in range(2):
                        drain_qkg(wn, tbs[ti], acc[(wn, ti)])
            # v+u rounds: 2 banks at a time (tags pa/pb rotate with groups: use
            # dedicated v tags on the tagset not used by the last group)
            vtags = TAGSETS[(2 * pair + 1) % 2][0:2]
            for ti in range(2):
                for half in range(2):
                    accv = [p1ps.tile([128, 272], F32, tag=vtags[i],
                                      name=f"accv{pair}{ti}{half}{i}")
                            for i in range(2)]
                    for d in range(ND):
                        for i in range(2):
                            nc.tensor.matmul(
                                accv[i][:],
                                hs_d(ti, d)[:, (2 * half + i) * 128:(2 * half + i + 1) * 128],
                                wts["wvu"][:, d, :],
                                start=(d == 0), stop=(d == ND - 1))
                    for i in range(2):
                        drain_vu(4 * tbs[ti] + 2 * half + i, accv[i])
            # slot chains: pair 0 inline; pair 1's both deferred to phase 2
            if pair == 0:
                for ti in range(2):
                    slot_chain(tbs[ti], mk_pt(p1ps, ("pb2", "pb3")), p1sb)
        p1ps_cm.__exit__(None, None, None)

    if debug:
        for h in range(2):
            nc.sync.dma_start(dbg["qT"].ap()[h * 128:(h + 1) * 128, :], qT[h][:])
            nc.sync.dma_start(dbg["kT"].ap()[h * 128:(h + 1) * 128, :], kT[h][:])
            nc.sync.dma_start(dbg["sg"].ap()[h * 128:(h + 1) * 128, :], sg[h][:])
        nc.sync.dma_start(dbg["v"].ap(), v[:].rearrange("p a b -> p (a b)"))
        nc.sync.dma_start(dbg["es"].ap(), es_t[:].rearrange("p a b -> p (a b)"))
        nc.sync.dma_start(dbg["enzT"].ap(), enzT[:])
        nc.sync.dma_start(dbg["esT"].ap(), esT[:])
        nc.sync.dma_start(dbg["u17"].ap(), u17[:])

    # ================= PHASE 2: attention =================
    with tc.tile_pool(name="p2sb", bufs=3) as p2sb, \
         tc.tile_pool(name="qveP", bufs=2) as qvep, \
         tc.tile_pool(name="dramp", bufs=1, space="DRAM") as dpool:
        ag_in, ag_out = [], []
        for h in range(2):
            ag_in.append(dpool.tile([N_CORES, 128, 256], BF16,
                                    tag=f"ag_in{h}", name=f"ag_in{h}"))
            ag_out.append(dpool.tile([N_CORES, 128, 256], BF16,
                                     tag=f"ag_out{h}", name=f"ag_out{h}"))
        warm_in = dpool.tile([N_CORES, 16], BF16, tag="warm_in", name="warm_in")
        warm_out = dpool.tile([N_CORES, 16], BF16, tag="warm_out", name="warm_out")
        wsrc = p2sb.tile([1, 16 * N_CORES], BF16, tag="wsrc", bufs=1)
        nc.vector.memset(wsrc[:], 0.0)
        nc.sync.dma_start(warm_in[:].rearrange("s t -> (s t)"), wsrc[0])
        epsb = p2sb.tile([128, 1], F32, tag="epsb", bufs=1)
        nc.vector.memset(epsb[:], EPS)
        tl = p2sb.tile(list(din["masks"].shape), BF16, tag="masks", name="masks", bufs=1)
        nc.sync.dma_start(tl[:], din["masks"].ap())
        c["masks"] = tl
        # o_proj weights: prefetch the full 8MB during attention
        wo_sb = p2sb.tile([128, ND, D], BF16, tag="wo_sb", name="wo_sb", bufs=1)
        nc.sync.dma_start(wo_sb[:], din["woT"].ap())

        with tc.tile_pool(name="ps_at", bufs=3, space="PSUM") as ps_at, \
             tc.tile_pool(name="ps_ok", bufs=2, space="PSUM") as ps_ok, \
             tc.tile_pool(name="ps_ot", bufs=2, space="PSUM") as ps_ot, \
             tc.tile_pool(name="ps_aux", bufs=1, space="PSUM") as ps_aux:
            def mk_pt2(pool):
                def pt(shape, name):
                    return pool.tile(shape, F32, tag="oT", name=name)
                return pt

            def stage1(I):
                rsl = slice(I * 512, (I + 1) * 512)
                njs = 4 * I + 4
                okT = ps_ok.tile([128, 512], F32, tag="okT", name=f"okT{I}",
                                 bufs=2)
                at_sb = {}

                def at_pair(j):
                    for h in range(2):
                        at = ps_at.tile([128, 512], F32, tag="atw2")
                        nc.tensor.matmul(at[:], kT[h][:, j * 128:(j + 1) * 128],
                                         qT[h][:, rsl], start=True, stop=True)
                        a_sb = p2sb.tile([128, 512], BF16, tag="at_sb")
                        m = j - 4 * I
                        if m >= 0:
                            nc.vector.tensor_mul(a_sb[:], at[:], c["masks"][:, m, :])
                        elif h == 0:
                            nc.scalar.copy(a_sb[:], at[:])
                        else:
                            nc.vector.tensor_copy(a_sb[:], at[:])
                        at_sb[(j, h)] = a_sb

                at_pair(0)
                for j in range(njs):
                    if j + 1 < njs:
                        at_pair(j + 1)
                    for h in range(2):
                        nc.tensor.matmul(okT[h * 64:(h + 1) * 64, :],
                                         es_t[:, j, h * 64:(h + 1) * 64],
                                         at_sb.pop((j, h))[:],
                                         start=(j == 0),
                                         stop=(j == njs - 1))
                return okT

            def softmax(I, okT):
                # unnormalized: qveT = exp(okT * enzT) * enzT  (RMS-norm
                # downstream cancels the per-(t,h) softmax denominator)
                rsl = slice(I * 512, (I + 1) * 512)
                qveT = qvep.tile([128, 512], BF16, tag="qveT")
                okm = p2sb.tile([128, 512], F32, tag="okm")
                nc.vector.tensor_mul(okm[:], okT[:], enzT[:, rsl])
                eok = p2sb.tile([128, 512], F32, tag="eok")
                nc.scalar.activation(eok[:], okm[:], AF.Exp)
                nc.vector.tensor_mul(qveT[:], eok[:], enzT[:, rsl])
                return qveT

            def stage2(I, qveT):
                rsl = slice(I * 512, (I + 1) * 512)
                njs = 4 * I + 4
                oT = [ps_ot.tile([128, 512], F32, tag="oT", name=f"oT{i}") for i in range(2)]
                w2_sb = {}

                def w2_pair(j):
                    for h in range(2):
                        w2 = ps_at.tile([128, 512], F32, tag="atw2")
                        nc.tensor.matmul(w2[:],
                                         esT[h * 64:(h + 1) * 64, j * 128:(j + 1) * 128],
                                         qveT[h * 64:(h + 1) * 64, :],
                                         start=True, stop=True)
                        wsb = p2sb.tile([128, 512], BF16, tag="at_sb")
                        m = j - 4 * I
                        if m >= 0:
                            nc.vector.tensor_mul(wsb[:], w2[:], c["masks"][:, m, :])
                        elif h == 0:
                            nc.scalar.copy(wsb[:], w2[:])
                        else:
                            nc.vector.tensor_copy(wsb[:], w2[:])
                        w2_sb[(j, h)] = wsb

                w2_pair(0)
                for j in range(njs):
                    if j + 1 < njs:
                        w2_pair(j + 1)
                    for h in range(2):
                        nc.tensor.matmul(oT[h][:], v[:, j, h * 128:(h + 1) * 128],
                                         w2_sb.pop((j, h))[:],
                                         start=(j == 0), stop=(j == njs - 1))
                # epilogue: o_g = o * rsqrt(mean o^2 + eps) * sg -> bf16 -> a2a_in
                for h in range(2):
                    sq = p2sb.tile([128, 512], F32R, tag="sq")
                    nc.scalar.activation(sq[:], oT[h][:], AF.Square)
                    ssq = ps_aux.tile([1, 512], F32, tag="aux")
                    nc.tensor.matmul(ssq[:], c["onescol_r"][:], sq[:], start=True, stop=True)
                    ssq_sb = p2sb.tile([1, 512], F32R, tag="ssq_sb")
                    with nc.allow_low_precision(reason="f32r bitcast for broadcast"):
                        nc.scalar.copy(ssq_sb[:], ssq[:])
                    rb = ps_aux.tile([128, 512], F32, tag="aux")
                    nc.tensor.matmul(rb[:], c["onesrow_r"][:], ssq_sb[:], start=True, stop=True)
                    rms = p2sb.tile([128, 512], F32, tag="rms")
                    nc.scalar.activation(rms[:], rb[:], AF.Sqrt, scale=1.0 / DV,
                                         bias=epsb[:])
                    rinv = p2sb.tile([128, 512], F32, tag="rinv")
                    nc.vector.reciprocal_approx_fast(rinv[:], rms[:])
                    t1 = p2sb.tile([128, 512], F32, tag="ept1")
                    nc.vector.tensor_mul(t1[:], oT[h][:], sg[h][:, rsl])
                    ogt = p2sb.tile([128, 512], BF16, tag="ogt")
                    nc.vector.tensor_mul(ogt[:], t1[:], rinv[:])
                    if debug:
                        nc.gpsimd.dma_start(dbg["ogT"].ap()[h * 128:(h + 1) * 128, rsl], ogt[:])
                    nc.sync.dma_start(ag_in[h][2 * I, :, :], ogt[:, 0:256])
                    nc.sync.dma_start(ag_in[h][2 * I + 1, :, :], ogt[:, 256:512])

            ok_prev = stage1(0)
            st["slot_chain"](NB - 2, mk_pt2(ps_ot), p2sb)
            st["slot_chain"](NB - 1, mk_pt2(ps_ot), p2sb)
            qv_prev = softmax(0, ok_prev)
            if debug:
                nc.sync.dma_start(dbg["qveT"].ap()[:, 0:512], qv_prev[:])
            for I in range(1, NB):
                okI = stage1(I)
                if I == 2:
                    # tiny warm-up A2A: keeps collective channels hot so the
                    # real transfers at the end run at steady-state latency
                    nc.gpsimd.collective_compute(
                        "AllToAll", mybir.AluOpType.bypass,
                        replica_groups=[list(range(N_CORES))],
                        ins=[warm_in[:].opt()], outs=[warm_out[:].opt()])
                qv_next = softmax(I, okI)
                stage2(I - 1, qv_prev)
                qv_prev = qv_next
                if debug:
                    nc.sync.dma_start(dbg["qveT"].ap()[:, I * 512:(I + 1) * 512], qv_prev[:])
            stage2(NB - 1, qv_prev)

        # ================= PHASE 3: per-head A2A + o_proj =================
        # A2A(h) starts once stage2(3)'s h epilogue DMAs land; o_proj h=0
        # matmuls overlap the h=1 transfer
        for h in range(2):
            nc.gpsimd.collective_compute(
                "AllToAll", mybir.AluOpType.bypass,
                replica_groups=[list(range(N_CORES))],
                ins=[ag_in[h][:].opt()], outs=[ag_out[h][:].opt()])
        og = {}
        for h in range(2):
            for s in range(N_CORES):
                ot = p2sb.tile([128, 256], BF16, tag=f"og{h}{s}",
                               name=f"og{h}{s}", bufs=1)
                nc.sync.dma_start(ot[:], ag_out[h][s, :, :])
                og[(h, s)] = ot
        p3ps_cm = tc.tile_pool(name="p3ps", bufs=1, space="PSUM")
        p3ps = p3ps_cm.__enter__()
        for th in range(2):
            pso = [p3ps.tile([128, 512], F32, tag=f"pso{th}{ns}",
                             name=f"pso{th}{ns}", bufs=1) for ns in range(4)]
            for n, (h, s) in enumerate((hh, ss) for hh in range(2)
                                       for ss in range(N_CORES)):
                kc = 2 * s + h
                for ns in range(4):
                    nc.tensor.matmul(pso[ns][:],
                                     og[(h, s)][:, th * 128:(th + 1) * 128],
                                     wo_sb[:, kc, ns * 512:(ns + 1) * 512],
                                     start=(n == 0), stop=(n == 15))
            for ns in range(4):
                osb = p2sb.tile([128, 512], F32, tag="osb")
                nc.scalar.copy(osb[:], pso[ns][:])
                nc.sync.dma_start(
                    out_d.ap()[th * 128:(th + 1) * 128, ns * 512:(ns + 1) * 512],
                    osb[:])
        p3ps_cm.__exit__(None, None, None)


# ======================= host side =======================

def _host_inputs(inputs):
    import ml_dtypes
    BF = ml_dtypes.bfloat16
    hs = np.ascontiguousarray(np.asarray(inputs["hidden_states"], np.float32)[0])
    Wq = np.asarray(inputs["Wq"], np.float32)
    Wk = np.asarray(inputs["Wk"], np.float32)
    Wv = np.asarray(inputs["Wv"], np.float32)
    Wg = np.asarray(inputs["Wg"], np.float32)
    Wo = np.asarray(inputs["Wo"], np.float32)
    Ws1 = np.asarray(inputs["Ws1"], np.float32)
    Ws2 = np.asarray(inputs["Ws2"], np.float32)
    bs2 = np.asarray(inputs["bs2"], np.float32)
    gnw = np.asarray(inputs["g_norm_weight"], np.float32)

    hsT = hs.T  # [D, T]
    # hsb: [p, chunk, k, t] with d = k*128 + p
    hsb = np.ascontiguousarray(
        hsT.reshape(ND, 128, NB, 512).transpose(1, 2, 0, 3)).astype(BF)
    pos = np.arange(T, dtype=np.float64)
    inv = 1.0 / (ROPE_BASE ** (np.arange(0, DK, 2, dtype=np.float64) / DK))
    ang = pos[:, None] * inv[None, :]
    cos = np.cos(ang).T.astype(np.float32)       # [64, T]
    sin = np.sin(ang).T.astype(np.float32)
    cossin = np.concatenate([cos, sin], axis=0).astype(np.float32)
    triu = np.triu(np.ones((128, 128), np.float32)).astype(BF)
    masks = np.zeros((128, 4, 512), np.float32)
    p = np.arange(128)[:, None]
    r = np.arange(512)[None, :]
    for m in range(4):
        masks[:, m, :] = (128 * m + p <= r).astype(np.float32)
    ident = np.eye(128, dtype=np.float32)
    onesrow = np.ones((1, 128), np.float32)
    ones2k = np.ones((1, T), np.float32).astype(BF)
    onescol = np.ones((128, 1), np.float32)
    # woT: [p, kc, n] with hd = kc*128 + p; gnw folded in
    woT = (Wo.T * np.tile(gnw, H)[:, None]).astype(BF)
    woT = np.ascontiguousarray(woT.reshape(ND, 128, D).transpose(1, 0, 2))

    def wlay(w):  # [2048, 128] -> [p, k, c] bf16
        return np.ascontiguousarray(
            w.reshape(ND, 128, -1).transpose(1, 0, 2)).astype(BF)

    in_maps = []
    for core in range(N_CORES):
        sl = slice(core * 256, (core + 1) * 256)
        ssl = slice(core * 128, (core + 1) * 128)
        ws2e = np.concatenate([Ws2[ssl].T, bs2[None, ssl]], axis=0).astype(BF)
        wvu = np.concatenate([Wv[sl].T, Ws1.T], axis=1)  # [2048, 272]
        m = {
            "hsb": hsb,
            "wq0": wlay(Wq[sl].T[:, 0:128] * SCALE),
            "wq1": wlay(Wq[sl].T[:, 128:256] * SCALE),
            "wk0": wlay(Wk[sl].T[:, 0:128]),
            "wk1": wlay(Wk[sl].T[:, 128:256]),
            "wg0": wlay(Wg[sl].T[:, 0:128]),
            "wg1": wlay(Wg[sl].T[:, 128:256]),
            "wvu": wlay(wvu),
            "ws2e": ws2e,
            "onesrow_b": onesrow.astype(BF), "onescol_b": onescol.astype(BF),
            "onesrow_r": onesrow, "onescol_r": onescol,
            "ones2k": ones2k,
            "cossin": cossin,
            "triu": triu, "masks": masks.astype(BF), "ident": ident,
            "woT": woT,
        }
        in_maps.append(m)
    return in_maps


_CACHE = {}


def kernel(**inputs):
    key = ("k", REPEAT, DEBUG)
    if key not in _CACHE:
        _CACHE[key] = build(repeat=REPEAT, debug=DEBUG)
    nc, dbg = _CACHE[key]
    in_maps = _host_inputs(inputs)
    res = bass_utils.run_bass_kernel_spmd(nc, in_maps, core_ids=list(range(N_CORES)))
    out = np.concatenate([res.results[c]["out"] for c in range(N_CORES)], axis=0)
    kernel.last_results = res
    return out.reshape(1, T, D).astype(np.float32)
